# revision 1
# baseline (speedup 1.0000x reference)
"""Trainium2 Bass kernel for a GPT-style transformer block (B=2, T=2048, C=1024, H=16).

Sharding: Megatron-style tensor parallelism over 8 NeuronCores.
  - Attention is head-parallel: each core computes QKV / attention for its 2 heads
    over all 4096 tokens.
  - A small AllToAll (1 MB/core) redistributes attention outputs from
    head-sharded to token-sharded.
  - The output projection, LayerNorm2 and the MLP are token-parallel: each core
    handles its 512-token shard with the full weight matrices.

Everything on-device runs in "transposed" layouts [feature, token] so that
weight matrices act as natural [in, out] stationary operands for the PE array
and per-feature biases broadcast along partitions. LayerNorm affine params are
folded into the weights on the host; the LN1 normalization itself is folded
around the QKV matmul as a per-token affine (raw @ W scaled post-hoc).
Softmax denominators come free from a ones-column appended to V.
"""

from contextlib import ExitStack

import numpy as np
import ml_dtypes

import concourse.bass as bass
import concourse.bacc as bacc
import concourse.mybir as mybir
import concourse.tile as tile
from concourse.bass_utils import run_bass_kernel_spmd

BF16 = mybir.dt.bfloat16
F32 = mybir.dt.float32
AF = mybir.ActivationFunctionType
OP = mybir.AluOpType

N_CORES = 8
B, T, C, H, D = 2, 2048, 1024, 16, 64
NTOK = B * T  # 4096
F = 4 * C  # 4096
LN_EPS = 1e-5
HPC = H // N_CORES  # heads per core = 2
SHARD = NTOK // N_CORES  # 512 tokens per core
NCH = C // 128  # 8 channel blocks
NFB = F // 128  # 32 ffn blocks
NCHUNK = NTOK // 512  # 8 token chunks of 512
SB = 128  # s-block size
VW = D + 1  # V block width incl. ones column = 65

_CACHE = {}



def _pbc(ap, n):
    """Partition-broadcast AP: read `ap` (1-D) n times across partitions."""
    return bass.AP(tensor=ap.tensor, offset=ap.offset,
                   ap=[[0, n]] + [list(x) for x in ap.ap])

_UID = [0]


def _t(pool, shape, dtype, tag):
    _UID[0] += 1
    return pool.tile(shape, dtype, tag=tag, name=f"{tag}_{_UID[0]}")


def _build(with_bias_qkv: bool):
    nc = bacc.Bacc("TRN2", target_bir_lowering=False, debug=False,
                   num_devices=N_CORES)

    # ---- I/O ----
    xT_d = nc.dram_tensor("xT", [C, NTOK], BF16, kind="ExternalInput")
    xTs_d = nc.dram_tensor("xTs", [C, SHARD], F32, kind="ExternalInput")
    wqkv_d = nc.dram_tensor("wqkv", [C, 3 * 128], BF16, kind="ExternalInput")
    wsum_d = nc.dram_tensor("wsum", [1, 3 * 128], BF16, kind="ExternalInput")
    bqkv_d = nc.dram_tensor("bqkv", [1, 3 * 128], BF16, kind="ExternalInput")
    wao_d = nc.dram_tensor("wao", [C, C], BF16, kind="ExternalInput")
    bao_d = nc.dram_tensor("bao", [C], F32, kind="ExternalInput")
    wfc_d = nc.dram_tensor("wfc", [C, F], BF16, kind="ExternalInput")
    bfc_d = nc.dram_tensor("bfc", [F], F32, kind="ExternalInput")
    wmp_d = nc.dram_tensor("wmp", [F, C], BF16, kind="ExternalInput")
    bmp_d = nc.dram_tensor("bmp", [C], F32, kind="ExternalInput")
    mask_d = nc.dram_tensor("mask", [128, 896], BF16, kind="ExternalInput")
    ident_d = nc.dram_tensor("ident", [128, 128], BF16, kind="ExternalInput")
    out_d = nc.dram_tensor("out", [C, SHARD], F32, kind="ExternalOutput")

    with tile.TileContext(nc) as tc, ExitStack() as _es:
            singles = _es.enter_context(tc.tile_pool(name="singles", bufs=1))
            dram = _es.enter_context(tc.tile_pool(name="dram", bufs=1, space="DRAM"))
            psp = _es.enter_context(tc.tile_pool(name="ps", bufs=2, space="PSUM"))
            psyp = _es.enter_context(tc.tile_pool(name="ps_y", bufs=2, space="PSUM"))
            pscp = _es.enter_context(tc.tile_pool(name="ps_c", bufs=2, space="PSUM"))
            # ---------- constants ----------
            eps_t = _t(singles, [128, 1], F32, "eps")
            nc.vector.memset(eps_t[:], LN_EPS)
            ones_t = _t(singles, [128, 1], BF16, "ones")
            nc.vector.memset(ones_t[:], 1.0)
            mask_t = _t(singles, [128, 896], BF16, "mask")
            nc.sync.dma_start(mask_t[:], mask_d[:])
            ident_t = _t(singles, [128, 128], BF16, "ident")
            nc.sync.dma_start(ident_t[:], ident_d[:])
            wsum_t = _t(singles, [1, 384], BF16, "wsum")
            nc.sync.dma_start(wsum_t[:], wsum_d[:])
            bqkv_col_t = _t(singles, [128, 3], F32, "bqkv_col")
            nc.gpsimd.dma_start(
                bqkv_col_t[:],
                bqkv_d.ap()[0, :].rearrange("(o p) -> p o", p=128))
            onesrow_t = _t(singles, [1, 512], BF16, "onesrow")
            nc.vector.memset(onesrow_t[:], 1.0)
            onesrow32_t = _t(singles, [1, 128], F32, "onesrow32")
            nc.vector.memset(onesrow32_t[:], 1.0)
            bao_t = _t(singles, [128, NCH], F32, "bao")
            nc.sync.dma_start(bao_t[:], bao_d.ap().rearrange("(o p) -> p o", p=128))
            bfc_t = _t(singles, [128, NFB], F32, "bfc")
            nc.sync.dma_start(bfc_t[:], bfc_d.ap().rearrange("(o p) -> p o", p=128))
            bmp_t = _t(singles, [128, NCH], F32, "bmp")
            nc.sync.dma_start(bmp_t[:], bmp_d.ap().rearrange("(o p) -> p o", p=128))

            # QKV weights resident: 3 out-blocks of [128ch x 8kb x 128o]
            wqkv_t = []
            for o in range(3):
                wt = _t(singles, [128, NCH, 128], BF16, f"wqkv{o}")
                nc.sync.dma_start(
                    wt[:],
                    wqkv_d.ap()[:, 128 * o:128 * (o + 1)].rearrange(
                        "(kb p) m -> p kb m", p=128),
                )
                wqkv_t.append(wt)

            # attention result tiles (resident, written per chunk)
            qT_t = _t(singles, [128, NTOK], BF16, "qT")
            kT_t = _t(singles, [128, NTOK], BF16, "kT")
            vT_t = _t(singles, [128, NTOK], BF16, "vT")
            qkv_dest = [qT_t, kT_t, vT_t]

            # A2A dram buffers (per shard: 2 heads x (64 y rows + denom row)).
            # Split in two collectives: batch-0 shards (slots 0-3) go in pair 1,
            # batch-1 shards in pair 2; unused slots stay zero so outputs sum.
            a2a_in1 = _t(dram, [N_CORES, HPC, VW, SHARD], BF16, "a2a_in1")
            a2a_out1 = _t(dram, [N_CORES, HPC, VW, SHARD], BF16, "a2a_out1")
            a2a_in2 = _t(dram, [N_CORES, HPC, VW, SHARD], BF16, "a2a_in2")
            a2a_out2 = _t(dram, [N_CORES, HPC, VW, SHARD], BF16, "a2a_out2")
            zt = _t(singles, [128, 520], BF16, "zt")
            nc.vector.memset(zt[:], 0.0)
            for i in range(4, 8):
                nc.sync.dma_start(a2a_in1[i].opt(), zt[:])
            for i in range(0, 4):
                nc.sync.dma_start(a2a_in2[i].opt(), zt[:])

            # ---- Phases A+B+C interleaved: LN1 stats + QKV + attention ----
            with ExitStack() as es1:
                pool_xt = es1.enter_context(tc.tile_pool(name="xt", bufs=8))
                pool_st = es1.enter_context(tc.tile_pool(name="st", bufs=4))
                pool_sqx = es1.enter_context(tc.tile_pool(name="sqx", bufs=4))
                pool_bc = es1.enter_context(tc.tile_pool(name="bc", bufs=8))
                pool_vt = es1.enter_context(tc.tile_pool(name="vt", bufs=2))
                pool_att = es1.enter_context(tc.tile_pool(name="att", bufs=4))
                pool_yt = es1.enter_context(tc.tile_pool(name="yt", bufs=4))

                xt_tiles = {}

                def stats_chunk(g):
                    t0 = 512 * g
                    xt = _t(pool_xt, [128, NCH, 512], BF16, "xt")
                    nc.sync.dma_start(
                        xt[:],
                        xT_d.ap()[:, t0:t0 + 512].rearrange(
                            "(kb p) t -> p kb t", p=128))
                    xt_tiles[g] = xt
                    # LN1 stats: sums of x and x^2 over channels via PE
                    s1p = _t(psp, [1, 512], F32, "ps")
                    s2p = _t(psp, [1, 512], F32, "ps")
                    for kb in range(NCH):
                        nc.tensor.matmul(s1p[:], ones_t[:], xt[:, kb, :],
                                         start=(kb == 0), stop=(kb == NCH - 1))
                    for kb in range(NCH):
                        sq = _t(pool_sqx, [128, 512], BF16, "sqx")
                        if kb % 2 == 0:
                            nc.scalar.activation(sq[:], xt[:, kb, :], AF.Square)
                        else:
                            nc.vector.tensor_tensor(sq[:], xt[:, kb, :],
                                                    xt[:, kb, :], OP.mult)
                        nc.tensor.matmul(s2p[:], ones_t[:], sq[:],
                                         start=(kb == 0), stop=(kb == NCH - 1))
                    nmu = _t(pool_bc, [1, 512], BF16, "nmu")
                    nc.vector.tensor_scalar(out=nmu[:], in0=s1p[:],
                                            scalar1=-1.0 / C, scalar2=0.0,
                                            op0=OP.mult, op1=OP.add)
                    mur = _t(pool_st, [1, 512], F32, "mur")
                    nc.vector.tensor_scalar(out=mur[:], in0=s1p[:],
                                            scalar1=1.0 / C, scalar2=0.0,
                                            op0=OP.mult, op1=OP.add)
                    msq = _t(pool_st, [1, 512], F32, "msq")
                    nc.vector.tensor_tensor(msq[:], mur[:], mur[:], OP.mult)
                    var = _t(pool_st, [1, 512], F32, "var")
                    nc.vector.scalar_tensor_tensor(
                        out=var[:], in0=s2p[:], scalar=1.0 / C, in1=msq[:],
                        op0=OP.mult, op1=OP.subtract)
                    srt = _t(pool_st, [1, 512], F32, "srt")
                    nc.scalar.activation(srt[:], var[:], AF.Sqrt,
                                         bias=eps_t[0:1, :], scale=1.0)
                    arow = _t(pool_st, [1, 512], F32, "arow")
                    nc.vector.reciprocal_approx_fast(arow[:], srt[:])
                    abp = _t(psp, [128, 512], F32, "ps")
                    nc.tensor.matmul(abp[:], onesrow32_t[:], arow[:])
                    abc = _t(pool_bc, [128, 512], F32, "abc")
                    nc.vector.tensor_copy(abc[:], abp[:])
                    return abc, nmu

                ab_tiles = {}

                def qkv_chunk(g):
                    t0 = 512 * g
                    xt = xt_tiles.pop(g)
                    abc, nmu = ab_tiles.pop(g)
                    for o in range(3):
                        ps = _t(psp, [128, 512], F32, "ps")
                        for kb in range(NCH):
                            nc.tensor.matmul(ps[:], wqkv_t[o][:, kb, :],
                                             xt[:, kb, :],
                                             start=(kb == 0), stop=False)
                        # rank-1 terms: wsum (x) b  [+ bias (x) ones]
                        nc.tensor.matmul(ps[:], wsum_t[:, 128 * o:128 * (o + 1)],
                                         nmu[:], start=False,
                                         stop=True)
                        dst = qkv_dest[o][:, t0:t0 + 512]
                        nc.vector.tensor_tensor(dst, ps[:], abc[:], OP.mult)
                        if with_bias_qkv:
                            nc.vector.tensor_scalar(
                                out=dst, in0=dst,
                                scalar1=bqkv_col_t[:, o:o + 1], scalar2=0.0,
                                op0=OP.add, op1=OP.add)

                vt = {}

                def att_chunk(b, j):
                    g = 4 * b + j
                    t0 = 512 * g
                    nblk = 4 * j + 4
                    if j == 0:
                        for h in range(HPC):
                            v = _t(pool_vt, [128, T // SB, VW], BF16, f"vt{h}")
                            nc.vector.memset(v[:, :, D:VW], 1.0)
                            vt[b, h] = v
                    # transpose V for the newly available s-blocks
                    for i in range(4 * j, 4 * j + 4):
                        s0 = 2048 * b + SB * i
                        for h in range(HPC):
                            pst = _t(psp, [128, D], BF16, "ps")
                            nc.tensor.transpose(
                                pst[:],
                                vT_t[64 * h:64 * (h + 1), s0:s0 + SB],
                                ident_t[64 * h:64 * (h + 1),
                                        64 * h:64 * (h + 1)],
                            )
                            nc.vector.tensor_copy(vt[b, h][:, i, 0:D], pst[:])
                    psy = [_t(psyp, [VW, 512], F32, "psy") for h in range(HPC)]
                    for i in range(nblk):
                        s0 = 2048 * b + SB * i
                        m = i - 4 * j  # >= 0 on diagonal blocks
                        f0 = 128 * m if m >= 0 else 0  # causal: t-f0 cols only
                        psc = _t(pscp, [128, HPC, 512], F32, "psc")
                        for h in range(HPC):
                            nc.tensor.matmul(
                                psc[:, h, 0:512 - f0],
                                kT_t[64 * h:64 * (h + 1), s0:s0 + SB],
                                qT_t[64 * h:64 * (h + 1), t0 + f0:t0 + 512],
                                tile_position=(64 * h, 0),
                            )
                        at = _t(pool_att, [128, HPC, 512], BF16, "att")
                        nc.scalar.activation(at[:, :, 0:512 - f0],
                                             psc[:, :, 0:512 - f0], AF.Exp)
                        if m >= 0:  # diagonal: mask boundary block only
                            for h in range(HPC):
                                nc.vector.tensor_tensor(
                                    at[:, h, 0:128], at[:, h, 0:128],
                                    mask_t[:, 384:512], OP.mult)
                        for h in range(HPC):
                            nc.tensor.matmul(
                                psy[h][:, f0:512], vt[b, h][:, i, :],
                                at[:, h, 0:512 - f0],
                                start=(i == 0), stop=(i == nblk - 1))
                    # ship raw y + softmax denominator row; normalize post-A2A
                    a2a_in = a2a_in1 if b == 0 else a2a_in2
                    for h in range(HPC):
                        yt = _t(pool_yt, [VW, 512], BF16, "yt")
                        nc.vector.tensor_copy(yt[:], psy[h][:])
                        nc.sync.dma_start(a2a_in[g, h], yt[:])

                for g in range(NCHUNK):
                    ab_tiles[g] = stats_chunk(g)
                for b in range(B):
                    for j in range(4):
                        qkv_chunk(4 * b + j)
                        att_chunk(b, j)
                    # AllToAll for this batch's shards overlaps the next batch
                    nc.gpsimd.collective_compute(
                        "AllToAll", OP.bypass,
                        replica_groups=[list(range(N_CORES))],
                        ins=[(a2a_in1 if b == 0 else a2a_in2).opt()],
                        outs=[(a2a_out1 if b == 0 else a2a_out2).opt()],
                    )

            # ---------- Phase D: AO proj + LN2 + MLP on the token shard ----
            with ExitStack() as es3:
                pool_x3 = es3.enter_context(tc.tile_pool(name="x3", bufs=1))
                pool_ya = es3.enter_context(tc.tile_pool(name="ya", bufs=2))
                pool_yn = es3.enter_context(tc.tile_pool(name="yn", bufs=8))
                pool_ao = es3.enter_context(tc.tile_pool(name="ao", bufs=8))
                pool_aob = es3.enter_context(tc.tile_pool(name="aob", bufs=8))
                pool_sq = es3.enter_context(tc.tile_pool(name="sq", bufs=2))
                pool_h2 = es3.enter_context(tc.tile_pool(name="h2", bufs=8))
                pool_mt = es3.enter_context(tc.tile_pool(name="mt", bufs=NFB))
                pool_w3 = es3.enter_context(tc.tile_pool(name="w3", bufs=3))
                pool_wm = es3.enter_context(tc.tile_pool(name="wm", bufs=2))
                pool_row2 = es3.enter_context(tc.tile_pool(name="row2", bufs=1))
                pool_bc2 = es3.enter_context(tc.tile_pool(name="bc2", bufs=1))
                pool_tmp2 = es3.enter_context(tc.tile_pool(name="tmp2", bufs=2))
                pool_ot = es3.enter_context(tc.tile_pool(name="ot", bufs=2))
                xts = _t(pool_x3, [128, NCH, 512], F32, "xts")
                nc.sync.dma_start(
                    xts[:], xTs_d.ap().rearrange("(kb p) t -> p kb t", p=128))
                yall = []
                for i in range(N_CORES):
                    yr1 = _t(pool_ya, [128, 512], BF16, "yr")
                    yr2 = _t(pool_ya, [128, 512], BF16, "yr")
                    dn1 = _t(pool_ya, [128, 512], F32, "dn")
                    dn2 = _t(pool_ya, [128, 512], F32, "dn")
                    for h in range(HPC):
                        nc.sync.dma_start(yr1[64 * h:64 * (h + 1), :],
                                          a2a_out1[i, h, 0:D, :])
                        nc.sync.dma_start(yr2[64 * h:64 * (h + 1), :],
                                          a2a_out2[i, h, 0:D, :])
                        nc.gpsimd.dma_start(
                            dn1[64 * h:64 * (h + 1), :],
                            _pbc(a2a_out1[i, h, D, :], D))
                        nc.gpsimd.dma_start(
                            dn2[64 * h:64 * (h + 1), :],
                            _pbc(a2a_out2[i, h, D, :], D))
                    yr = _t(pool_ya, [128, 512], BF16, "yrs")
                    nc.vector.tensor_tensor(yr[:], yr1[:], yr2[:], OP.add)
                    dn = _t(pool_ya, [128, 512], F32, "dns")
                    nc.vector.tensor_tensor(dn[:], dn1[:], dn2[:], OP.add)
                    dr = _t(pool_ya, [128, 512], F32, "dr")
                    nc.vector.reciprocal_approx_fast(dr[:], dn[:])
                    ya = _t(pool_yn, [128, 512], BF16, "ya")
                    nc.vector.tensor_tensor(ya[:], yr[:], dr[:], OP.mult)
                    yall.append(ya)
                aot, aob = [], []
                for w in range(NCH):
                    wt = _t(pool_w3, [128, NCH, 128], BF16, "w3")
                    nc.sync.dma_start(
                        wt[:],
                        wao_d.ap()[:, 128 * w:128 * (w + 1)].rearrange(
                            "(kb p) m -> p kb m", p=128))
                    ps = _t(psp, [128, 512], F32, "ps")
                    for i in range(NCH):
                        nc.tensor.matmul(ps[:], wt[:, i, :], yall[i][:],
                                         start=(i == 0), stop=(i == NCH - 1))
                    ao = _t(pool_ao, [128, 512], F32, "ao")
                    nc.vector.scalar_tensor_tensor(
                        out=ao[:], in0=ps[:], scalar=bao_t[:, w:w + 1],
                        in1=xts[:, w, :], op0=OP.add, op1=OP.add)
                    ab = _t(pool_aob, [128, 512], BF16, "aob")
                    nc.vector.tensor_copy(ab[:], ao[:])
                    aot.append(ao)
                    aob.append(ab)
                # LN2 stats over channels via ones-matmuls
                ps1 = _t(psyp, [1, 512], F32, "psy")
                for w in range(NCH):
                    nc.tensor.matmul(ps1[:], ones_t[:], aob[w][:],
                                     start=(w == 0), stop=(w == NCH - 1))
                ps2 = _t(psyp, [1, 512], F32, "psy")
                for w in range(NCH):
                    sq = _t(pool_sq, [128, 512], BF16, "sq")
                    nc.vector.tensor_tensor(sq[:], aob[w][:], aob[w][:], OP.mult)
                    nc.tensor.matmul(ps2[:], ones_t[:], sq[:],
                                     start=(w == 0), stop=(w == NCH - 1))
                mur = _t(pool_row2, [1, 512], F32, "mur")
                nc.vector.tensor_scalar(out=mur[:], in0=ps1[:], scalar1=1.0 / C,
                                        scalar2=0.0, op0=OP.mult, op1=OP.add)
                e2r = _t(pool_row2, [1, 512], F32, "e2r")
                nc.vector.tensor_scalar(out=e2r[:], in0=ps2[:], scalar1=1.0 / C,
                                        scalar2=0.0, op0=OP.mult, op1=OP.add)
                msq = _t(pool_row2, [1, 512], F32, "msq")
                nc.vector.tensor_tensor(msq[:], mur[:], mur[:], OP.mult)
                varr = _t(pool_row2, [1, 512], F32, "varr")
                nc.vector.tensor_tensor(varr[:], e2r[:], msq[:], OP.subtract)
                srow2 = _t(pool_row2, [1, 512], F32, "srow2")
                nc.scalar.activation(srow2[:], varr[:], AF.Sqrt,
                                     bias=eps_t[0:1, :], scale=1.0)
                rrow2 = _t(pool_row2, [1, 512], F32, "rrow2")
                nc.vector.reciprocal_approx_fast(rrow2[:], srow2[:])
                mup = _t(psp, [128, 512], F32, "ps")
                nc.tensor.matmul(mup[:], onesrow32_t[:], mur[:])
                mubc = _t(pool_bc2, [128, 512], F32, "mubc")
                nc.vector.tensor_copy(mubc[:], mup[:])
                rbp = _t(psp, [128, 512], F32, "ps")
                nc.tensor.matmul(rbp[:], onesrow32_t[:], rrow2[:])
                rbc2 = _t(pool_bc2, [128, 512], F32, "rbc2")
                nc.vector.tensor_copy(rbc2[:], rbp[:])
                h2 = []
                for w in range(NCH):
                    tp = _t(pool_tmp2, [128, 512], F32, "tmp2")
                    nc.vector.tensor_tensor(tp[:], aot[w][:], mubc[:], OP.subtract)
                    ht = _t(pool_h2, [128, 512], BF16, "h2")
                    nc.vector.tensor_tensor(ht[:], tp[:], rbc2[:], OP.mult)
                    h2.append(ht)
                # FC + GELU
                mt = []
                for fg in range(NFB // 4):
                    wt = _t(pool_w3, [128, NCH, 4, 128], BF16, "w3")
                    nc.sync.dma_start(
                        wt[:],
                        wfc_d.ap()[:, 512 * fg:512 * (fg + 1)].rearrange(
                            "(kb p) (fs m) -> p kb fs m", p=128, m=128))
                    for fs in range(4):
                        f = 4 * fg + fs
                        ps = _t(psp, [128, 512], F32, "ps")
                        for cb in range(NCH):
                            nc.tensor.matmul(ps[:], wt[:, cb, fs, :], h2[cb][:],
                                             start=(cb == 0),
                                             stop=(cb == NCH - 1))
                        m = _t(pool_mt, [128, 512], BF16, "mt")
                        nc.scalar.activation(m[:], ps[:], AF.Gelu,
                                             bias=bfc_t[:, f:f + 1], scale=1.0)
                        mt.append(m)
                # MP + bias + residual -> out
                for w in range(NCH):
                    wt = _t(pool_wm, [128, NFB, 128], BF16, "wm")
                    nc.sync.dma_start(
                        wt[:],
                        wmp_d.ap().rearrange("(fb p) o -> p fb o", p=128)[
                            :, :, 128 * w:128 * (w + 1)])
                    ps = _t(psp, [128, 512], F32, "ps")
                    for f in range(NFB):
                        nc.tensor.matmul(ps[:], wt[:, f, :], mt[f][:],
                                         start=(f == 0), stop=(f == NFB - 1))
                    ot = _t(pool_ot, [128, 512], F32, "ot")
                    nc.vector.scalar_tensor_tensor(
                        out=ot[:], in0=ps[:], scalar=bmp_t[:, w:w + 1],
                        in1=aot[w][:], op0=OP.add, op1=OP.add)
                    nc.sync.dma_start(out_d[128 * w:128 * (w + 1), :], ot[:])

    nc.compile()
    return nc


def _prep(inputs):
    """Host-side preprocessing: fold LN affines into weights, slice per core."""
    f32 = np.float32
    bf16 = ml_dtypes.bfloat16
    x = np.asarray(inputs["x"], f32).reshape(NTOK, C)
    W_qkv = np.asarray(inputs["W_qkv"], f32)
    b_qkv = np.asarray(inputs["b_qkv"], f32)
    W_ao = np.asarray(inputs["W_ao"], f32)
    b_ao = np.asarray(inputs["b_ao"], f32)
    W_fc = np.asarray(inputs["W_fc"], f32)
    b_fc = np.asarray(inputs["b_fc"], f32)
    W_mp = np.asarray(inputs["W_mp"], f32)
    b_mp = np.asarray(inputs["b_mp"], f32)
    g1 = np.asarray(inputs["g1"], f32)
    be1 = np.asarray(inputs["be1"], f32)
    g2 = np.asarray(inputs["g2"], f32)
    be2 = np.asarray(inputs["be2"], f32)

    Wq_eff = W_qkv * g1[:, None]
    bq_eff = b_qkv + be1 @ W_qkv
    # fold 1/sqrt(D) into the Q columns
    Wq_eff[:, :C] *= 1.0 / np.sqrt(D)
    bq_eff[:C] *= 1.0 / np.sqrt(D)
    Wfc_eff = W_fc * g2[:, None]
    bfc_eff = b_fc + be2 @ W_fc

    xT = np.ascontiguousarray(x.T)
    xT_bf = xT.astype(bf16)
    mask = (np.arange(128)[:, None] <= (np.arange(896)[None, :] - 384)).astype(bf16)
    ident = np.eye(128, dtype=bf16)

    wao_bf = W_ao.astype(bf16)
    wfc_bf = Wfc_eff.astype(bf16)
    wmp_bf = W_mp.astype(bf16)

    with_bias_qkv = bool(np.any(bq_eff != 0.0))

    in_maps = []
    for r in range(N_CORES):
        cs = 128 * r
        wq_core = np.concatenate(
            [Wq_eff[:, cs:cs + 128], Wq_eff[:, C + cs:C + cs + 128],
             Wq_eff[:, 2 * C + cs:2 * C + cs + 128]], axis=1)
        bq_core = np.concatenate(
            [bq_eff[cs:cs + 128], bq_eff[C + cs:C + cs + 128],
             bq_eff[2 * C + cs:2 * C + cs + 128]])
        wsum_core = wq_core.sum(axis=0).astype(f32)
        in_maps.append({
            "xT": xT_bf,
            "xTs": np.ascontiguousarray(xT[:, SHARD * r:SHARD * (r + 1)]),
            "wqkv": wq_core.astype(bf16),
            "wsum": np.ascontiguousarray(wsum_core).astype(bf16).reshape(1, -1),
            "bqkv": np.ascontiguousarray(bq_core).astype(bf16).reshape(1, -1),
            "wao": wao_bf,
            "bao": b_ao,
            "wfc": wfc_bf,
            "bfc": bfc_eff.astype(f32),
            "wmp": wmp_bf,
            "bmp": b_mp,
            "mask": mask,
            "ident": ident,
        })
    return in_maps, with_bias_qkv


def kernel(_trace=False, _trace_kwargs=None, **inputs):
    in_maps, with_bias_qkv = _prep(inputs)
    key = ("nc", with_bias_qkv)
    if key not in _CACHE:
        _CACHE[key] = _build(with_bias_qkv)
    nc = _CACHE[key]
    res = run_bass_kernel_spmd(
        nc, in_maps, core_ids=list(range(N_CORES)),
        trace=_trace, **(_trace_kwargs or {}))
    _CACHE["last_results"] = res
    out = np.concatenate(
        [np.asarray(res.results[r]["out"]).T for r in range(N_CORES)], axis=0)
    return np.ascontiguousarray(out.reshape(B, T, C)).astype(np.float32)



# revision 22
# speedup vs baseline: 1.3020x; 1.3020x over previous
"""Trainium2 Bass kernel for a GPT-style transformer block (B=2, T=2048, C=1024, H=16).

Sharding: Megatron-style tensor parallelism over 8 NeuronCores.
  - Attention is head-parallel: each core computes QKV / attention for its 2 heads
    over all 4096 tokens.
  - Attention outputs are softmax-normalized sender-side, then two half-size
    AllToAlls (one per batch) redistribute them from head-sharded to
    token-sharded. Each core's token shard interleaves both batches
    (256 tokens of batch 0 + 256 of batch 1) so neither collective ships
    zero padding.
  - The output projection, LayerNorm2 and the MLP are token-parallel: each core
    handles its 512-token shard with the full weight matrices.

Everything on-device runs in "transposed" layouts [feature, token] so that
weight matrices act as natural [in, out] stationary operands for the PE array
and per-feature biases broadcast along partitions. LayerNorm affine params are
folded into the weights on the host; the LN normalizations are folded around
the matmuls as per-token affines computed from broadcasted statistics
(ones[128,128] stationary matmuls give partition-broadcast sums directly).
All weights are laid out host-side so every DMA descriptor is >=8KB
contiguous per partition.
"""

from contextlib import ExitStack

import numpy as np
import ml_dtypes

import concourse.bass as bass
import concourse.bacc as bacc
import concourse.mybir as mybir
import concourse.tile as tile
from concourse.bass_utils import run_bass_kernel_spmd

BF16 = mybir.dt.bfloat16
F32 = mybir.dt.float32
AF = mybir.ActivationFunctionType
OP = mybir.AluOpType

N_CORES = 8
B, T, C, H, D = 2, 2048, 1024, 16, 64
NTOK = B * T  # 4096
F = 4 * C  # 4096
LN_EPS = 1e-5
HPC = H // N_CORES  # heads per core = 2
SHARD = NTOK // N_CORES  # 512 tokens per core (256 from each batch)
HALF = SHARD // 2  # 256
NCH = C // 128  # 8 channel blocks
NFB = F // 128  # 32 ffn blocks
NCHUNK = NTOK // 512  # 8 token chunks of 512
SB = 128  # s-block size
VW = D + 1  # V block width incl. ones column = 65

_CACHE = {}
_DEBUG = False


def _pbc(ap, n):
    """Partition-broadcast AP: read `ap` (1-D) n times across partitions."""
    return bass.AP(tensor=ap.tensor, offset=ap.offset,
                   ap=[[0, n]] + [list(x) for x in ap.ap])


_UID = [0]


def _t(pool, shape, dtype, tag):
    _UID[0] += 1
    return pool.tile(shape, dtype, tag=tag, name=f"{tag}_{_UID[0]}")


def _build(with_bias_qkv: bool):
    nc = bacc.Bacc("TRN2", target_bir_lowering=False, debug=False,
                   num_devices=N_CORES)

    # ---- I/O ----  (all weight tensors pre-laid-out host side, partition-major)
    xTr_d = nc.dram_tensor("xTr", [128, NCHUNK, NCH, 512], BF16,
                           kind="ExternalInput")
    xts_d = nc.dram_tensor("xts", [128, NCH, SHARD], F32, kind="ExternalInput")
    wqkv_d = nc.dram_tensor("wqkv", [128, 3, NCH, 128], BF16,
                            kind="ExternalInput")
    wsum_d = nc.dram_tensor("wsum", [128, 3], F32, kind="ExternalInput")
    bqkv_d = nc.dram_tensor("bqkv", [128, 3], F32, kind="ExternalInput")
    wao_d = nc.dram_tensor("wao", [128, NCH, NCH, 128], BF16,
                           kind="ExternalInput")
    bao_d = nc.dram_tensor("bao", [128, NCH], F32, kind="ExternalInput")
    wfc_d = nc.dram_tensor("wfc", [128, NFB, NCH, 128], BF16,
                           kind="ExternalInput")
    bfc_d = nc.dram_tensor("bfc", [128, NFB], F32, kind="ExternalInput")
    wmp_d = nc.dram_tensor("wmp", [128, NCH, NFB, 128], BF16,
                           kind="ExternalInput")
    bmp_d = nc.dram_tensor("bmp", [128, NCH], F32, kind="ExternalInput")
    mask_d = nc.dram_tensor("mask", [128, 128], BF16, kind="ExternalInput")
    ident_d = nc.dram_tensor("ident", [128, 128], BF16, kind="ExternalInput")
    out_d = nc.dram_tensor("out", [C, SHARD], F32, kind="ExternalOutput")
    if _DEBUG:
        dbg_q_d = nc.dram_tensor("dbg_q", [128, 512], BF16,
                                 kind="ExternalOutput")
        dbg_yt_d = nc.dram_tensor("dbg_yt", [VW, 512], BF16,
                                  kind="ExternalOutput")
        dbg_ya_d = nc.dram_tensor("dbg_ya", [128, SHARD], BF16,
                                  kind="ExternalOutput")
        dbg_ao_d = nc.dram_tensor("dbg_ao", [128, SHARD], F32,
                                  kind="ExternalOutput")

    with tile.TileContext(nc) as tc, ExitStack() as _es:
        singles = _es.enter_context(tc.tile_pool(name="singles", bufs=1))
        dram = _es.enter_context(tc.tile_pool(name="dram", bufs=1, space="DRAM"))
        # ---------- constants ----------
        eps_t = _t(singles, [128, 1], F32, "eps")
        nc.vector.memset(eps_t[:], LN_EPS)
        ones128_t = _t(singles, [128, 128], BF16, "ones128")
        nc.vector.memset(ones128_t[:], 1.0)
        mask_t = _t(singles, [128, 128], BF16, "mask")
        nc.sync.dma_start(mask_t[:], mask_d[:])
        ident_t = _t(singles, [128, 128], BF16, "ident")
        nc.sync.dma_start(ident_t[:], ident_d[:])
        wsum_t = _t(singles, [128, 3], F32, "wsum")
        nc.sync.dma_start(wsum_t[:], wsum_d[:])
        bqkv_t = _t(singles, [128, 3], F32, "bqkv")
        nc.sync.dma_start(bqkv_t[:], bqkv_d[:])
        bao_t = _t(singles, [128, NCH], F32, "bao")
        nc.sync.dma_start(bao_t[:], bao_d[:])
        bfc_t = _t(singles, [128, NFB], F32, "bfc")
        nc.sync.dma_start(bfc_t[:], bfc_d[:])
        bmp_t = _t(singles, [128, NCH], F32, "bmp")
        nc.sync.dma_start(bmp_t[:], bmp_d[:])

        # QKV weights resident: [128, 3, NCH, 128]
        wqkv_t = _t(singles, [128, 3, NCH, 128], BF16, "wqkv")
        nc.sync.dma_start(wqkv_t[:], wqkv_d[:])
        # AO weights resident, prefetched early (used in phase D)
        wao_t = _t(singles, [128, NCH, NCH, 128], BF16, "wao")
        nc.sync.dma_start(wao_t[:], wao_d[:])

        # A2A dram buffers: per batch, slot i carries the 256-token half-shard
        # i (tokens [256*i, 256*i+256) of that batch) for this core's 2 heads:
        # 64 raw y rows + the softmax denominator row. No zero slots.
        a2a_in1 = _t(dram, [N_CORES, HPC, VW, HALF], BF16, "a2a_in1")
        a2a_out1 = _t(dram, [N_CORES, HPC, VW, HALF], BF16, "a2a_out1")
        a2a_in2 = _t(dram, [N_CORES, HPC, VW, HALF], BF16, "a2a_in2")
        a2a_out2 = _t(dram, [N_CORES, HPC, VW, HALF], BF16, "a2a_out2")

        # ---- Phase ABC: LN1 stats + QKV + attention, chunk-pipelined ----
        with ExitStack() as es1:
            ps_sh = es1.enter_context(
                tc.tile_pool(name="ps_sh", bufs=2, space="PSUM"))
            pscp = es1.enter_context(
                tc.tile_pool(name="ps_c", bufs=2, space="PSUM"))
            psyp = es1.enter_context(
                tc.tile_pool(name="ps_y", bufs=2, space="PSUM"))
            pool_xt = es1.enter_context(tc.tile_pool(name="xt", bufs=3))
            pool_st = es1.enter_context(tc.tile_pool(name="st", bufs=4))
            pool_ab = es1.enter_context(tc.tile_pool(name="ab", bufs=4))
            pool_sqx = es1.enter_context(tc.tile_pool(name="sqx", bufs=4))
            pool_tmp = es1.enter_context(tc.tile_pool(name="tmpq", bufs=2))
            pool_vt = es1.enter_context(tc.tile_pool(name="vt", bufs=2))
            pool_att = es1.enter_context(tc.tile_pool(name="att", bufs=4))
            pool_yt = es1.enter_context(tc.tile_pool(name="yt", bufs=4))
            pool_qkvT = es1.enter_context(tc.tile_pool(name="qkvT", bufs=1))

            # attention result tiles (live through phase ABC only)
            qT_t = _t(pool_qkvT, [128, NTOK], BF16, "qT")
            kT_t = _t(pool_qkvT, [128, NTOK], BF16, "kT")
            vT_t = _t(pool_qkvT, [128, NTOK], BF16, "vT")
            qkv_dest = [qT_t, kT_t, vT_t]

            vt = {}

            def do_chunk(b, j):
                g = 4 * b + j
                t0 = 512 * g
                # ---- load x chunk (fat descriptors: 8KB/partition) ----
                xt = _t(pool_xt, [128, NCH, 512], BF16, "xt")
                nc.sync.dma_start(xt[:], xTr_d.ap()[:, g])
                # ---- LN1 stats via broadcast matmuls (M=128) ----
                s1p = _t(ps_sh, [128, 512], F32, "ps")
                for kb in range(NCH):
                    nc.tensor.matmul(s1p[:], ones128_t[:], xt[:, kb, :],
                                     start=(kb == 0), stop=(kb == NCH - 1))
                s2p = _t(ps_sh, [128, 512], F32, "ps")
                for kb in range(NCH):
                    sq = _t(pool_sqx, [128, 512], BF16, "sqx")
                    if kb % 2 == 0:
                        nc.scalar.activation(sq[:], xt[:, kb, :], AF.Square)
                    else:
                        nc.vector.tensor_tensor(sq[:], xt[:, kb, :],
                                                xt[:, kb, :], OP.mult)
                    nc.tensor.matmul(s2p[:], ones128_t[:], sq[:],
                                     start=(kb == 0), stop=(kb == NCH - 1))
                # broadcast rows: nmu = -mean, abc = 1/sqrt(var+eps)
                nmu = _t(pool_ab, [128, 512], BF16, "nmu")
                nc.vector.tensor_scalar(out=nmu[:], in0=s1p[:],
                                        scalar1=-1.0 / C, scalar2=0.0,
                                        op0=OP.mult, op1=OP.add)
                mur = _t(pool_st, [128, 512], F32, "mur")
                nc.vector.tensor_scalar(out=mur[:], in0=s1p[:],
                                        scalar1=1.0 / C, scalar2=0.0,
                                        op0=OP.mult, op1=OP.add)
                msq = _t(pool_st, [128, 512], F32, "msq")
                nc.vector.tensor_tensor(msq[:], mur[:], mur[:], OP.mult)
                var = _t(pool_st, [128, 512], F32, "var")
                nc.vector.scalar_tensor_tensor(
                    out=var[:], in0=s2p[:], scalar=1.0 / C, in1=msq[:],
                    op0=OP.mult, op1=OP.subtract)
                srt = _t(pool_st, [128, 512], F32, "srt")
                nc.scalar.activation(srt[:], var[:], AF.Sqrt,
                                     bias=eps_t[:], scale=1.0)
                abc = _t(pool_ab, [128, 512], F32, "abc")
                nc.vector.reciprocal_approx_fast(abc[:], srt[:])
                # ---- QKV on raw x + rank-1 LN correction on vector ----
                for o in range(3):
                    ps = _t(ps_sh, [128, 512], F32, "ps")
                    for kb in range(NCH):
                        nc.tensor.matmul(ps[:], wqkv_t[:, o, kb, :],
                                         xt[:, kb, :],
                                         start=(kb == 0), stop=(kb == NCH - 1))
                    tmp = _t(pool_tmp, [128, 512], F32, "tmpq")
                    nc.vector.scalar_tensor_tensor(
                        out=tmp[:], in0=nmu[:], scalar=wsum_t[:, o:o + 1],
                        in1=ps[:], op0=OP.mult, op1=OP.add)
                    dst = qkv_dest[o][:, t0:t0 + 512]
                    nc.vector.tensor_tensor(dst, tmp[:], abc[:], OP.mult)
                    if _DEBUG and g == 0 and o == 0:
                        nc.sync.dma_start(dbg_q_d[:], dst)
                    if with_bias_qkv:
                        nc.vector.tensor_scalar(
                            out=dst, in0=dst,
                            scalar1=bqkv_t[:, o:o + 1], scalar2=0.0,
                            op0=OP.add, op1=OP.add)
                # ---- attention for this chunk ----
                nblk = 4 * j + 4
                if j == 0:
                    for h in range(HPC):
                        v = _t(pool_vt, [128, T // SB, VW], BF16, f"vt{h}")
                        nc.vector.memset(v[:, :, D:VW], 1.0)
                        vt[b, h] = v
                # transpose V for the newly available s-blocks
                for i in range(4 * j, 4 * j + 4):
                    s0 = 2048 * b + SB * i
                    for h in range(HPC):
                        pst = _t(ps_sh, [128, D], BF16, "ps")
                        nc.tensor.transpose(
                            pst[:],
                            vT_t[64 * h:64 * (h + 1), s0:s0 + SB],
                            ident_t[64 * h:64 * (h + 1),
                                    64 * h:64 * (h + 1)],
                        )
                        nc.vector.tensor_copy(vt[b, h][:, i, 0:D], pst[:])
                psy = [_t(psyp, [VW, 512], F32, "psy") for h in range(HPC)]
                for i in range(nblk):
                    s0 = 2048 * b + SB * i
                    m = i - 4 * j  # >= 0 on diagonal blocks
                    f0 = 128 * m if m >= 0 else 0  # causal: t-f0 cols only
                    psc = _t(pscp, [128, HPC, 512], F32, "psc")
                    for h in range(HPC):
                        nc.tensor.matmul(
                            psc[:, h, 0:512 - f0],
                            kT_t[64 * h:64 * (h + 1), s0:s0 + SB],
                            qT_t[64 * h:64 * (h + 1), t0 + f0:t0 + 512],
                            tile_position=(64 * h, 0),
                        )
                    at = _t(pool_att, [128, HPC, 512], BF16, "att")
                    nc.scalar.activation(at[:, :, 0:512 - f0],
                                         psc[:, :, 0:512 - f0], AF.Exp)
                    if m >= 0:  # diagonal: mask boundary block only
                        for h in range(HPC):
                            nc.vector.tensor_tensor(
                                at[:, h, 0:128], at[:, h, 0:128],
                                mask_t[:], OP.mult)
                    for h in range(HPC):
                        nc.tensor.matmul(
                            psy[h][:, f0:512], vt[b, h][:, i, :],
                            at[:, h, 0:512 - f0],
                            start=(i == 0), stop=(i == nblk - 1))
                # ship raw y + denominator row; normalize post-A2A
                a2a_in = a2a_in1 if b == 0 else a2a_in2
                for h in range(HPC):
                    yt = _t(pool_yt, [VW, 512], BF16, "yt")
                    nc.vector.tensor_copy(yt[:], psy[h][:])
                    if _DEBUG and g == 0 and h == 0:
                        nc.sync.dma_start(dbg_yt_d[:], yt[:])
                    nc.sync.dma_start(a2a_in[2 * j, h], yt[:, 0:HALF])
                    nc.sync.dma_start(a2a_in[2 * j + 1, h], yt[:, HALF:512])

            for b in range(B):
                for j in range(4):
                    do_chunk(b, j)
                nc.gpsimd.collective_compute(
                    "AllToAll", OP.bypass,
                    replica_groups=[list(range(N_CORES))],
                    ins=[(a2a_in1 if b == 0 else a2a_in2).opt()],
                    outs=[(a2a_out1 if b == 0 else a2a_out2).opt()],
                )

        # ---------- Phase D: AO proj + LN2 + MLP on the token shard ----
        with ExitStack() as es3:
            psd = es3.enter_context(
                tc.tile_pool(name="psd", bufs=4, space="PSUM"))
            psr = es3.enter_context(
                tc.tile_pool(name="psr", bufs=2, space="PSUM"))
            pool_x3 = es3.enter_context(tc.tile_pool(name="x3", bufs=1))
            pool_ya = es3.enter_context(tc.tile_pool(name="ya", bufs=8))
            pool_yn = es3.enter_context(tc.tile_pool(name="yn", bufs=2))
            pool_ao = es3.enter_context(tc.tile_pool(name="ao", bufs=8))
            pool_aob = es3.enter_context(tc.tile_pool(name="aob", bufs=8))
            pool_sq = es3.enter_context(tc.tile_pool(name="sq", bufs=2))
            pool_st2 = es3.enter_context(tc.tile_pool(name="st2", bufs=4))
            pool_h2 = es3.enter_context(tc.tile_pool(name="h2", bufs=8))
            pool_mt = es3.enter_context(tc.tile_pool(name="mt", bufs=NFB))
            pool_wf = es3.enter_context(tc.tile_pool(name="wf", bufs=2))
            pool_wm = es3.enter_context(tc.tile_pool(name="wm", bufs=2))
            pool_ot = es3.enter_context(tc.tile_pool(name="ot", bufs=2))

            xts = _t(pool_x3, [128, NCH, SHARD], F32, "xts")
            nc.sync.dma_start(xts[:], xts_d[:])
            # assemble y: batch-0 half in cols 0:256, batch-1 half in 256:512,
            # then normalize by the broadcast softmax denominators
            yall = []
            for i in range(N_CORES):
                yr = _t(pool_yn, [128, SHARD], BF16, "yr")
                dn = _t(pool_yn, [128, SHARD], F32, "dn")
                for h in range(HPC):
                    nc.sync.dma_start(yr[64 * h:64 * (h + 1), 0:HALF],
                                      a2a_out1[i, h, 0:D, :])
                    nc.sync.dma_start(yr[64 * h:64 * (h + 1), HALF:SHARD],
                                      a2a_out2[i, h, 0:D, :])
                    nc.gpsimd.dma_start(
                        dn[64 * h:64 * (h + 1), 0:HALF],
                        _pbc(a2a_out1[i, h, D, :], D))
                    nc.gpsimd.dma_start(
                        dn[64 * h:64 * (h + 1), HALF:SHARD],
                        _pbc(a2a_out2[i, h, D, :], D))
                rdn = _t(pool_yn, [128, SHARD], F32, "rdn")
                nc.vector.reciprocal_approx_fast(rdn[:], dn[:])
                ya = _t(pool_ya, [128, SHARD], BF16, "ya")
                nc.vector.tensor_tensor(ya[:], yr[:], rdn[:], OP.mult)
                yall.append(ya)
            if _DEBUG:
                nc.sync.dma_start(dbg_ya_d[:], yall[0][:])
            aot, aob = [], []
            for w in range(NCH):
                ps = _t(psd, [128, SHARD], F32, "psd")
                for i in range(NCH):
                    nc.tensor.matmul(ps[:], wao_t[:, w, i, :], yall[i][:],
                                     start=(i == 0), stop=(i == NCH - 1))
                ao = _t(pool_ao, [128, SHARD], F32, "ao")
                nc.vector.scalar_tensor_tensor(
                    out=ao[:], in0=ps[:], scalar=bao_t[:, w:w + 1],
                    in1=xts[:, w, :], op0=OP.add, op1=OP.add)
                ab = _t(pool_aob, [128, SHARD], BF16, "aob")
                nc.vector.tensor_copy(ab[:], ao[:])
                if _DEBUG and w == 0:
                    nc.sync.dma_start(dbg_ao_d[:], ao[:])
                aot.append(ao)
                aob.append(ab)
            # LN2 stats via broadcast matmuls
            ps1 = _t(psr, [128, SHARD], F32, "psr")
            for w in range(NCH):
                nc.tensor.matmul(ps1[:], ones128_t[:], aob[w][:],
                                 start=(w == 0), stop=(w == NCH - 1))
            ps2 = _t(psr, [128, SHARD], F32, "psr")
            for w in range(NCH):
                sq = _t(pool_sq, [128, SHARD], BF16, "sq")
                if w % 2 == 0:
                    nc.scalar.activation(sq[:], aob[w][:], AF.Square)
                else:
                    nc.vector.tensor_tensor(sq[:], aob[w][:], aob[w][:],
                                            OP.mult)
                nc.tensor.matmul(ps2[:], ones128_t[:], sq[:],
                                 start=(w == 0), stop=(w == NCH - 1))
            mur = _t(pool_st2, [128, SHARD], F32, "mur2")
            nc.vector.tensor_scalar(out=mur[:], in0=ps1[:], scalar1=1.0 / C,
                                    scalar2=0.0, op0=OP.mult, op1=OP.add)
            msq = _t(pool_st2, [128, SHARD], F32, "msq2")
            nc.vector.tensor_tensor(msq[:], mur[:], mur[:], OP.mult)
            var = _t(pool_st2, [128, SHARD], F32, "var2")
            nc.vector.scalar_tensor_tensor(
                out=var[:], in0=ps2[:], scalar=1.0 / C, in1=msq[:],
                op0=OP.mult, op1=OP.subtract)
            srt = _t(pool_st2, [128, SHARD], F32, "srt2")
            nc.scalar.activation(srt[:], var[:], AF.Sqrt,
                                 bias=eps_t[:], scale=1.0)
            rbc = _t(pool_st2, [128, SHARD], F32, "rbc2")
            nc.vector.reciprocal_approx_fast(rbc[:], srt[:])
            h2 = []
            for w in range(NCH):
                tp = _t(pool_sq, [128, SHARD], F32, "tmp2")
                nc.vector.tensor_tensor(tp[:], aot[w][:], mur[:], OP.subtract)
                ht = _t(pool_h2, [128, SHARD], BF16, "h2")
                nc.vector.tensor_tensor(ht[:], tp[:], rbc[:], OP.mult)
                h2.append(ht)
            # FC + GELU (stream weights, 8KB/partition descriptors)
            mt = []
            for fg in range(NFB // 4):
                wt = _t(pool_wf, [128, 4, NCH, 128], BF16, "wf")
                nc.sync.dma_start(wt[:], wfc_d.ap()[:, 4 * fg:4 * fg + 4])
                for fs in range(4):
                    f = 4 * fg + fs
                    ps = _t(psd, [128, SHARD], F32, "psd")
                    for cb in range(NCH):
                        nc.tensor.matmul(ps[:], wt[:, fs, cb, :], h2[cb][:],
                                         start=(cb == 0),
                                         stop=(cb == NCH - 1))
                    m = _t(pool_mt, [128, SHARD], BF16, "mt")
                    nc.scalar.activation(m[:], ps[:], AF.Gelu,
                                         bias=bfc_t[:, f:f + 1], scale=1.0)
                    mt.append(m)
            # MP + bias + residual -> out
            for w in range(NCH):
                wt = _t(pool_wm, [128, NFB, 128], BF16, "wm")
                nc.sync.dma_start(wt[:], wmp_d.ap()[:, w])
                ps = _t(psd, [128, SHARD], F32, "psd")
                for f in range(NFB):
                    nc.tensor.matmul(ps[:], wt[:, f, :], mt[f][:],
                                     start=(f == 0), stop=(f == NFB - 1))
                ot = _t(pool_ot, [128, SHARD], F32, "ot")
                nc.vector.scalar_tensor_tensor(
                    out=ot[:], in0=ps[:], scalar=bmp_t[:, w:w + 1],
                    in1=aot[w][:], op0=OP.add, op1=OP.add)
                nc.sync.dma_start(out_d[128 * w:128 * (w + 1), :], ot[:])

    nc.compile()
    return nc


def _prep(inputs):
    """Host-side preprocessing: fold LN affines into weights, relayout."""
    f32 = np.float32
    bf16 = ml_dtypes.bfloat16
    x = np.asarray(inputs["x"], f32).reshape(NTOK, C)
    W_qkv = np.asarray(inputs["W_qkv"], f32)
    b_qkv = np.asarray(inputs["b_qkv"], f32)
    W_ao = np.asarray(inputs["W_ao"], f32)
    b_ao = np.asarray(inputs["b_ao"], f32)
    W_fc = np.asarray(inputs["W_fc"], f32)
    b_fc = np.asarray(inputs["b_fc"], f32)
    W_mp = np.asarray(inputs["W_mp"], f32)
    b_mp = np.asarray(inputs["b_mp"], f32)
    g1 = np.asarray(inputs["g1"], f32)
    be1 = np.asarray(inputs["be1"], f32)
    g2 = np.asarray(inputs["g2"], f32)
    be2 = np.asarray(inputs["be2"], f32)

    Wq_eff = W_qkv * g1[:, None]
    bq_eff = b_qkv + be1 @ W_qkv
    # fold 1/sqrt(D) into the Q columns
    Wq_eff[:, :C] *= 1.0 / np.sqrt(D)
    bq_eff[:C] *= 1.0 / np.sqrt(D)
    Wfc_eff = W_fc * g2[:, None]
    bfc_eff = b_fc + be2 @ W_fc

    xT = np.ascontiguousarray(x.T)  # [C, NTOK]
    # [128, chunk, kb, 512]: partition-major, 8KB contiguous per (p, chunk)
    xTr = np.ascontiguousarray(
        xT.astype(bf16).reshape(NCH, 128, NCHUNK, 512).transpose(1, 2, 0, 3))
    mask = (np.arange(128)[:, None] <= np.arange(128)[None, :]).astype(bf16)
    ident = np.eye(128, dtype=bf16)

    # wao: [128, w, kb, 128]
    waor = np.ascontiguousarray(
        W_ao.astype(bf16).reshape(NCH, 128, NCH, 128).transpose(1, 2, 0, 3))
    # wfc: [128, f, kb, 128]
    wfcr = np.ascontiguousarray(
        Wfc_eff.astype(bf16).reshape(NCH, 128, NFB, 128).transpose(1, 2, 0, 3))
    # wmp: [128, w, fb, 128]
    wmpr = np.ascontiguousarray(
        W_mp.astype(bf16).reshape(NFB, 128, NCH, 128).transpose(1, 2, 0, 3))
    bao_c = np.ascontiguousarray(b_ao.reshape(NCH, 128).T)
    bfc_c = np.ascontiguousarray(bfc_eff.reshape(NFB, 128).T)
    bmp_c = np.ascontiguousarray(b_mp.reshape(NCH, 128).T)

    with_bias_qkv = bool(np.any(bq_eff != 0.0))

    in_maps = []
    for r in range(N_CORES):
        cs = 128 * r
        # per-core qkv column block: [C, 3, 128] -> [128, 3, kb, 128]
        wq_core = np.stack(
            [Wq_eff[:, o * C + cs:o * C + cs + 128] for o in range(3)],
            axis=1)  # [C, 3, 128]
        wq_bf = wq_core.astype(bf16)
        wqkvr = np.ascontiguousarray(
            wq_bf.reshape(NCH, 128, 3, 128).transpose(1, 2, 0, 3))
        # exact colsums of the bf16 weights actually used on device
        wsum_core = wq_bf.astype(f32).sum(axis=0)  # [3, 128]
        bq_core = np.stack(
            [bq_eff[o * C + cs:o * C + cs + 128] for o in range(3)], axis=0)
        # interleaved shard: 256 tokens of batch 0 + 256 of batch 1
        cols = np.concatenate([
            np.arange(HALF * r, HALF * r + HALF),
            np.arange(T + HALF * r, T + HALF * r + HALF)])
        xts_core = np.ascontiguousarray(
            xT[:, cols].reshape(NCH, 128, SHARD).transpose(1, 0, 2))
        in_maps.append({
            "xTr": xTr,
            "xts": xts_core,
            "wqkv": wqkvr,
            "wsum": np.ascontiguousarray(wsum_core.T),
            "bqkv": np.ascontiguousarray(bq_core.T),
            "wao": waor,
            "bao": bao_c,
            "wfc": wfcr,
            "bfc": bfc_c,
            "wmp": wmpr,
            "bmp": bmp_c,
            "mask": mask,
            "ident": ident,
        })
    return in_maps, with_bias_qkv


def kernel(_trace=False, _trace_kwargs=None, **inputs):
    in_maps, with_bias_qkv = _prep(inputs)
    key = ("nc", with_bias_qkv)
    if key not in _CACHE:
        _CACHE[key] = _build(with_bias_qkv)
    nc = _CACHE[key]
    res = run_bass_kernel_spmd(
        nc, in_maps, core_ids=list(range(N_CORES)),
        trace=_trace, **(_trace_kwargs or {}))
    _CACHE["last_results"] = res
    # un-shard: core r holds [C, 512] = [b0 tokens 256r:256r+256 | b1 same]
    out_full = np.empty((C, NTOK), dtype=np.float32)
    for r in range(N_CORES):
        o = np.asarray(res.results[r]["out"])
        out_full[:, HALF * r:HALF * r + HALF] = o[:, :HALF]
        out_full[:, T + HALF * r:T + HALF * r + HALF] = o[:, HALF:]
    return np.ascontiguousarray(out_full.T.reshape(B, T, C)).astype(np.float32)


# revision 23
# speedup vs baseline: 1.3023x; 1.0003x over previous
"""Trainium2 Bass kernel for a GPT-style transformer block (B=2, T=2048, C=1024, H=16).

Sharding: Megatron-style tensor parallelism over 8 NeuronCores.
  - Attention is head-parallel: each core computes QKV / attention for its 2 heads
    over all 4096 tokens.
  - Two half-size AllToAlls (one per batch) redistribute attention outputs from
    head-sharded to token-sharded. Each core's token shard interleaves both
    batches (256 tokens of batch 0 + 256 of batch 1) so neither collective
    ships zero padding, and the batch-0 half is assembled under batch-1 compute.
  - The output projection, LayerNorm2 and the MLP are token-parallel: each core
    handles its 512-token shard with the full weight matrices.

Everything on-device runs in "transposed" layouts [feature, token]. LayerNorm
affine params are folded into the weights on the host; the LN normalizations
are folded around the matmuls as per-token affines computed from broadcasted
statistics (ones[128,128] stationary matmuls give partition-broadcast sums
directly). LN stats for all 4 chunks of a batch are computed up-front so the
scalar engine's activation table switches (Sqrt vs Exp) happen once per batch
instead of per chunk. All weights are laid out host-side so every DMA
descriptor is >=8KB contiguous per partition.
"""

from contextlib import ExitStack

import numpy as np
import ml_dtypes

import concourse.bass as bass
import concourse.bacc as bacc
import concourse.mybir as mybir
import concourse.tile as tile
from concourse.bass_utils import run_bass_kernel_spmd

BF16 = mybir.dt.bfloat16
F32 = mybir.dt.float32
AF = mybir.ActivationFunctionType
OP = mybir.AluOpType

N_CORES = 8
B, T, C, H, D = 2, 2048, 1024, 16, 64
NTOK = B * T  # 4096
F = 4 * C  # 4096
LN_EPS = 1e-5
HPC = H // N_CORES  # heads per core = 2
SHARD = NTOK // N_CORES  # 512 tokens per core (256 from each batch)
HALF = SHARD // 2  # 256
NCH = C // 128  # 8 channel blocks
NFB = F // 128  # 32 ffn blocks
NCHUNK = NTOK // 512  # 8 token chunks of 512
SB = 128  # s-block size
VW = D + 1  # V block width incl. ones column = 65

_CACHE = {}
_DEBUG = False


def _pbc(ap, n):
    """Partition-broadcast AP: read `ap` (1-D) n times across partitions."""
    return bass.AP(tensor=ap.tensor, offset=ap.offset,
                   ap=[[0, n]] + [list(x) for x in ap.ap])


_UID = [0]


def _t(pool, shape, dtype, tag):
    _UID[0] += 1
    return pool.tile(shape, dtype, tag=tag, name=f"{tag}_{_UID[0]}")


def _build(with_bias_qkv: bool):
    nc = bacc.Bacc("TRN2", target_bir_lowering=False, debug=False,
                   num_devices=N_CORES)

    # ---- I/O ----  (all weight tensors pre-laid-out host side, partition-major)
    xTr_d = nc.dram_tensor("xTr", [128, NCHUNK, NCH, 512], BF16,
                           kind="ExternalInput")
    xts_d = nc.dram_tensor("xts", [128, NCH, SHARD], F32, kind="ExternalInput")
    wqkv_d = nc.dram_tensor("wqkv", [128, 3, NCH, 128], BF16,
                            kind="ExternalInput")
    wsum_d = nc.dram_tensor("wsum", [128, 3], F32, kind="ExternalInput")
    bqkv_d = nc.dram_tensor("bqkv", [128, 3], F32, kind="ExternalInput")
    wao_d = nc.dram_tensor("wao", [128, NCH, NCH, 128], BF16,
                           kind="ExternalInput")
    bao_d = nc.dram_tensor("bao", [128, NCH], F32, kind="ExternalInput")
    wfc_d = nc.dram_tensor("wfc", [128, NFB, NCH, 128], BF16,
                           kind="ExternalInput")
    bfc_d = nc.dram_tensor("bfc", [128, NFB], F32, kind="ExternalInput")
    wmp_d = nc.dram_tensor("wmp", [128, NCH, NFB, 128], BF16,
                           kind="ExternalInput")
    bmp_d = nc.dram_tensor("bmp", [128, NCH], F32, kind="ExternalInput")
    mask_d = nc.dram_tensor("mask", [128, 128], BF16, kind="ExternalInput")
    ident_d = nc.dram_tensor("ident", [128, 128], BF16, kind="ExternalInput")
    out_d = nc.dram_tensor("out", [C, SHARD], F32, kind="ExternalOutput")
    if _DEBUG:
        dbg_q_d = nc.dram_tensor("dbg_q", [128, 512], BF16,
                                 kind="ExternalOutput")
        dbg_yt_d = nc.dram_tensor("dbg_yt", [VW, 512], BF16,
                                  kind="ExternalOutput")
        dbg_ya_d = nc.dram_tensor("dbg_ya", [128, SHARD], BF16,
                                  kind="ExternalOutput")
        dbg_ao_d = nc.dram_tensor("dbg_ao", [128, SHARD], F32,
                                  kind="ExternalOutput")

    with tile.TileContext(nc) as tc, ExitStack() as _es:
        singles = _es.enter_context(tc.tile_pool(name="singles", bufs=1))
        dram = _es.enter_context(tc.tile_pool(name="dram", bufs=1, space="DRAM"))
        # y-assembly pools live across both phases: the batch-0 half is
        # assembled under batch-1 attention compute.
        pool_ya = _es.enter_context(tc.tile_pool(name="ya", bufs=8))
        pool_yn = _es.enter_context(tc.tile_pool(name="yn", bufs=2))
        # ---------- constants (tiny DMAs first; big weights deferred) ------
        eps_t = _t(singles, [128, 1], F32, "eps")
        nc.vector.memset(eps_t[:], LN_EPS)
        ones128_t = _t(singles, [128, 128], BF16, "ones128")
        nc.vector.memset(ones128_t[:], 1.0)
        mask_t = _t(singles, [128, 128], BF16, "mask")
        nc.sync.dma_start(mask_t[:], mask_d[:])
        ident_t = _t(singles, [128, 128], BF16, "ident")
        nc.sync.dma_start(ident_t[:], ident_d[:])
        wsum_t = _t(singles, [128, 3], F32, "wsum")
        nc.sync.dma_start(wsum_t[:], wsum_d[:])
        bqkv_t = _t(singles, [128, 3], F32, "bqkv")
        nc.sync.dma_start(bqkv_t[:], bqkv_d[:])
        bao_t = _t(singles, [128, NCH], F32, "bao")
        nc.sync.dma_start(bao_t[:], bao_d[:])
        bfc_t = _t(singles, [128, NFB], F32, "bfc")
        nc.sync.dma_start(bfc_t[:], bfc_d[:])
        bmp_t = _t(singles, [128, NCH], F32, "bmp")
        nc.sync.dma_start(bmp_t[:], bmp_d[:])
        # big resident weights; DMAs issued later to keep startup lean
        wqkv_t = _t(singles, [128, 3, NCH, 128], BF16, "wqkv")
        wao_t = _t(singles, [128, NCH, NCH, 128], BF16, "wao")

        # A2A dram buffers: per batch, slot i carries the 256-token half-shard
        # i (tokens [256*i, 256*i+256) of that batch) for this core's 2 heads:
        # 64 raw y rows + the softmax denominator row. No zero slots.
        a2a_in1 = _t(dram, [N_CORES, HPC, VW, HALF], BF16, "a2a_in1")
        a2a_out1 = _t(dram, [N_CORES, HPC, VW, HALF], BF16, "a2a_out1")
        a2a_in2 = _t(dram, [N_CORES, HPC, VW, HALF], BF16, "a2a_in2")
        a2a_out2 = _t(dram, [N_CORES, HPC, VW, HALF], BF16, "a2a_out2")

        yall = [None] * N_CORES

        def assemble_half(i, half, a2a_out):
            """Normalize source core i's y for one 256-token half-shard."""
            if yall[i] is None:
                yall[i] = _t(pool_ya, [128, SHARD], BF16, "ya")
            c0, c1 = (0, HALF) if half == 0 else (HALF, SHARD)
            yr = _t(pool_yn, [128, HALF], BF16, "yr")
            dn = _t(pool_yn, [128, HALF], F32, "dn")
            for h in range(HPC):
                nc.sync.dma_start(yr[64 * h:64 * (h + 1), :],
                                  a2a_out[i, h, 0:D, :])
                nc.gpsimd.dma_start(dn[64 * h:64 * (h + 1), :],
                                    _pbc(a2a_out[i, h, D, :], D))
            rdn = _t(pool_yn, [128, HALF], F32, "rdn")
            nc.vector.reciprocal_approx_fast(rdn[:], dn[:])
            nc.vector.tensor_tensor(yall[i][:, c0:c1], yr[:], rdn[:], OP.mult)

        # ---- Phase ABC: LN1 stats + QKV + attention, batch-pipelined ----
        with ExitStack() as es1:
            ps_sh = es1.enter_context(
                tc.tile_pool(name="ps_sh", bufs=2, space="PSUM"))
            pscp = es1.enter_context(
                tc.tile_pool(name="ps_c", bufs=2, space="PSUM"))
            psyp = es1.enter_context(
                tc.tile_pool(name="ps_y", bufs=2, space="PSUM"))
            pool_xt = es1.enter_context(tc.tile_pool(name="xt", bufs=5))
            pool_st = es1.enter_context(tc.tile_pool(name="st", bufs=4))
            pool_ab = es1.enter_context(tc.tile_pool(name="ab", bufs=5))
            pool_sqx = es1.enter_context(tc.tile_pool(name="sqx", bufs=4))
            pool_tmp = es1.enter_context(tc.tile_pool(name="tmpq", bufs=2))
            pool_vt = es1.enter_context(tc.tile_pool(name="vt", bufs=2))
            pool_att = es1.enter_context(tc.tile_pool(name="att", bufs=4))
            pool_yt = es1.enter_context(tc.tile_pool(name="yt", bufs=4))
            pool_qkvT = es1.enter_context(tc.tile_pool(name="qkvT", bufs=1))

            # attention result tiles (live through phase ABC only)
            qT_t = _t(pool_qkvT, [128, NTOK], BF16, "qT")
            kT_t = _t(pool_qkvT, [128, NTOK], BF16, "kT")
            vT_t = _t(pool_qkvT, [128, NTOK], BF16, "vT")
            qkv_dest = [qT_t, kT_t, vT_t]

            vt = {}
            xt_tiles = {}
            ab_tiles = {}

            def stats_chunk(b, j):
                g = 4 * b + j
                # ---- load x chunk (fat descriptors: 8KB/partition) ----
                xt = _t(pool_xt, [128, NCH, 512], BF16, "xt")
                nc.sync.dma_start(xt[:], xTr_d.ap()[:, g])
                xt_tiles[g] = xt
                if b == 0 and j == 0:
                    # big weight prefetches ride behind the first chunk
                    nc.sync.dma_start(wqkv_t[:], wqkv_d[:])
                if b == 0 and j == 3:
                    nc.sync.dma_start(wao_t[:], wao_d[:])
                # ---- LN1 stats via broadcast matmuls (M=128) ----
                s1p = _t(ps_sh, [128, 512], F32, "ps")
                for kb in range(NCH):
                    nc.tensor.matmul(s1p[:], ones128_t[:], xt[:, kb, :],
                                     start=(kb == 0), stop=(kb == NCH - 1))
                s2p = _t(ps_sh, [128, 512], F32, "ps")
                for kb in range(NCH):
                    sq = _t(pool_sqx, [128, 512], BF16, "sqx")
                    nc.vector.tensor_tensor(sq[:], xt[:, kb, :],
                                            xt[:, kb, :], OP.mult)
                    nc.tensor.matmul(s2p[:], ones128_t[:], sq[:],
                                     start=(kb == 0), stop=(kb == NCH - 1))
                # broadcast rows: nmu = -mean, abc = 1/sqrt(var+eps)
                nmu = _t(pool_ab, [128, 512], BF16, "nmu")
                nc.vector.tensor_scalar(out=nmu[:], in0=s1p[:],
                                        scalar1=-1.0 / C, scalar2=0.0,
                                        op0=OP.mult, op1=OP.add)
                mur = _t(pool_st, [128, 512], F32, "mur")
                nc.vector.tensor_scalar(out=mur[:], in0=s1p[:],
                                        scalar1=1.0 / C, scalar2=0.0,
                                        op0=OP.mult, op1=OP.add)
                msq = _t(pool_st, [128, 512], F32, "msq")
                nc.vector.tensor_tensor(msq[:], mur[:], mur[:], OP.mult)
                var = _t(pool_st, [128, 512], F32, "var")
                nc.vector.scalar_tensor_tensor(
                    out=var[:], in0=s2p[:], scalar=1.0 / C, in1=msq[:],
                    op0=OP.mult, op1=OP.subtract)
                srt = _t(pool_st, [128, 512], F32, "srt")
                nc.scalar.activation(srt[:], var[:], AF.Sqrt,
                                     bias=eps_t[:], scale=1.0)
                abc = _t(pool_ab, [128, 512], F32, "abc")
                nc.vector.reciprocal_approx_fast(abc[:], srt[:])
                ab_tiles[g] = (nmu, abc)

            def att_chunk(b, j):
                g = 4 * b + j
                t0 = 512 * g
                # ---- QKV on raw x + rank-1 LN correction on vector ----
                xt = xt_tiles.pop(g)
                nmu, abc = ab_tiles.pop(g)
                for o in range(3):
                    ps = _t(ps_sh, [128, 512], F32, "ps")
                    for kb in range(NCH):
                        nc.tensor.matmul(ps[:], wqkv_t[:, o, kb, :],
                                         xt[:, kb, :],
                                         start=(kb == 0), stop=(kb == NCH - 1))
                    tmp = _t(pool_tmp, [128, 512], F32, "tmpq")
                    nc.vector.scalar_tensor_tensor(
                        out=tmp[:], in0=nmu[:], scalar=wsum_t[:, o:o + 1],
                        in1=ps[:], op0=OP.mult, op1=OP.add)
                    dst = qkv_dest[o][:, t0:t0 + 512]
                    nc.vector.tensor_tensor(dst, tmp[:], abc[:], OP.mult)
                    if _DEBUG and g == 0 and o == 0:
                        nc.sync.dma_start(dbg_q_d[:], dst)
                    if with_bias_qkv:
                        nc.vector.tensor_scalar(
                            out=dst, in0=dst,
                            scalar1=bqkv_t[:, o:o + 1], scalar2=0.0,
                            op0=OP.add, op1=OP.add)
                # ---- attention for this chunk ----
                nblk = 4 * j + 4
                if j == 0:
                    for h in range(HPC):
                        v = _t(pool_vt, [128, T // SB, VW], BF16, f"vt{h}")
                        nc.vector.memset(v[:, :, D:VW], 1.0)
                        vt[b, h] = v
                # transpose V for the newly available s-blocks
                for i in range(4 * j, 4 * j + 4):
                    s0 = 2048 * b + SB * i
                    for h in range(HPC):
                        pst = _t(ps_sh, [128, D], BF16, "ps")
                        nc.tensor.transpose(
                            pst[:],
                            vT_t[64 * h:64 * (h + 1), s0:s0 + SB],
                            ident_t[64 * h:64 * (h + 1),
                                    64 * h:64 * (h + 1)],
                        )
                        nc.vector.tensor_copy(vt[b, h][:, i, 0:D], pst[:])
                psy = [_t(psyp, [VW, 512], F32, "psy") for h in range(HPC)]
                for i in range(nblk):
                    s0 = 2048 * b + SB * i
                    m = i - 4 * j  # >= 0 on diagonal blocks
                    f0 = 128 * m if m >= 0 else 0  # causal: t-f0 cols only
                    psc = _t(pscp, [128, HPC, 512], F32, "psc")
                    for h in range(HPC):
                        nc.tensor.matmul(
                            psc[:, h, 0:512 - f0],
                            kT_t[64 * h:64 * (h + 1), s0:s0 + SB],
                            qT_t[64 * h:64 * (h + 1), t0 + f0:t0 + 512],
                            tile_position=(64 * h, 0),
                        )
                    at = _t(pool_att, [128, HPC, 512], BF16, "att")
                    nc.scalar.activation(at[:, :, 0:512 - f0],
                                         psc[:, :, 0:512 - f0], AF.Exp)
                    if m >= 0:  # diagonal: mask boundary block only
                        for h in range(HPC):
                            nc.vector.tensor_tensor(
                                at[:, h, 0:128], at[:, h, 0:128],
                                mask_t[:], OP.mult)
                    for h in range(HPC):
                        nc.tensor.matmul(
                            psy[h][:, f0:512], vt[b, h][:, i, :],
                            at[:, h, 0:512 - f0],
                            start=(i == 0), stop=(i == nblk - 1))
                # ship raw y + denominator row; normalize post-A2A
                a2a_in = a2a_in1 if b == 0 else a2a_in2
                for h in range(HPC):
                    yt = _t(pool_yt, [VW, 512], BF16, "yt")
                    nc.vector.tensor_copy(yt[:], psy[h][:])
                    if _DEBUG and g == 0 and h == 0:
                        nc.sync.dma_start(dbg_yt_d[:], yt[:])
                    nc.sync.dma_start(a2a_in[2 * j, h], yt[:, 0:HALF])
                    nc.sync.dma_start(a2a_in[2 * j + 1, h], yt[:, HALF:512])

            for b in range(B):
                for j in range(4):
                    stats_chunk(b, j)
                for j in range(4):
                    att_chunk(b, j)
                nc.gpsimd.collective_compute(
                    "AllToAll", OP.bypass,
                    replica_groups=[list(range(N_CORES))],
                    ins=[(a2a_in1 if b == 0 else a2a_in2).opt()],
                    outs=[(a2a_out1 if b == 0 else a2a_out2).opt()],
                )
                if b == 0:
                    # assemble the batch-0 half under batch-1 compute
                    for i in range(N_CORES):
                        assemble_half(i, 0, a2a_out1)

        # ---------- Phase D: AO proj + LN2 + MLP on the token shard ----
        with ExitStack() as es3:
            psd = es3.enter_context(
                tc.tile_pool(name="psd", bufs=4, space="PSUM"))
            psr = es3.enter_context(
                tc.tile_pool(name="psr", bufs=2, space="PSUM"))
            pool_x3 = es3.enter_context(tc.tile_pool(name="x3", bufs=1))
            pool_ao = es3.enter_context(tc.tile_pool(name="ao", bufs=8))
            pool_aob = es3.enter_context(tc.tile_pool(name="aob", bufs=8))
            pool_sq = es3.enter_context(tc.tile_pool(name="sq", bufs=2))
            pool_st2 = es3.enter_context(tc.tile_pool(name="st2", bufs=4))
            pool_h2 = es3.enter_context(tc.tile_pool(name="h2", bufs=8))
            pool_mt = es3.enter_context(tc.tile_pool(name="mt", bufs=NFB))
            pool_wf = es3.enter_context(tc.tile_pool(name="wf", bufs=2))
            pool_wm = es3.enter_context(tc.tile_pool(name="wm", bufs=2))
            pool_ot = es3.enter_context(tc.tile_pool(name="ot", bufs=2))

            xts = _t(pool_x3, [128, NCH, SHARD], F32, "xts")
            nc.sync.dma_start(xts[:], xts_d[:])
            # assemble the batch-1 half of y
            for i in range(N_CORES):
                assemble_half(i, 1, a2a_out2)
            if _DEBUG:
                nc.sync.dma_start(dbg_ya_d[:], yall[0][:])
            aot, aob = [], []
            for w in range(NCH):
                ps = _t(psd, [128, SHARD], F32, "psd")
                for i in range(NCH):
                    nc.tensor.matmul(ps[:], wao_t[:, w, i, :], yall[i][:],
                                     start=(i == 0), stop=(i == NCH - 1))
                ao = _t(pool_ao, [128, SHARD], F32, "ao")
                nc.vector.scalar_tensor_tensor(
                    out=ao[:], in0=ps[:], scalar=bao_t[:, w:w + 1],
                    in1=xts[:, w, :], op0=OP.add, op1=OP.add)
                ab = _t(pool_aob, [128, SHARD], BF16, "aob")
                nc.vector.tensor_copy(ab[:], ao[:])
                if _DEBUG and w == 0:
                    nc.sync.dma_start(dbg_ao_d[:], ao[:])
                aot.append(ao)
                aob.append(ab)
            # LN2 stats via broadcast matmuls
            ps1 = _t(psr, [128, SHARD], F32, "psr")
            for w in range(NCH):
                nc.tensor.matmul(ps1[:], ones128_t[:], aob[w][:],
                                 start=(w == 0), stop=(w == NCH - 1))
            ps2 = _t(psr, [128, SHARD], F32, "psr")
            for w in range(NCH):
                sq = _t(pool_sq, [128, SHARD], BF16, "sq")
                nc.vector.tensor_tensor(sq[:], aob[w][:], aob[w][:],
                                        OP.mult)
                nc.tensor.matmul(ps2[:], ones128_t[:], sq[:],
                                 start=(w == 0), stop=(w == NCH - 1))
            mur = _t(pool_st2, [128, SHARD], F32, "mur2")
            nc.vector.tensor_scalar(out=mur[:], in0=ps1[:], scalar1=1.0 / C,
                                    scalar2=0.0, op0=OP.mult, op1=OP.add)
            msq = _t(pool_st2, [128, SHARD], F32, "msq2")
            nc.vector.tensor_tensor(msq[:], mur[:], mur[:], OP.mult)
            var = _t(pool_st2, [128, SHARD], F32, "var2")
            nc.vector.scalar_tensor_tensor(
                out=var[:], in0=ps2[:], scalar=1.0 / C, in1=msq[:],
                op0=OP.mult, op1=OP.subtract)
            srt = _t(pool_st2, [128, SHARD], F32, "srt2")
            nc.scalar.activation(srt[:], var[:], AF.Sqrt,
                                 bias=eps_t[:], scale=1.0)
            rbc = _t(pool_st2, [128, SHARD], F32, "rbc2")
            nc.vector.reciprocal_approx_fast(rbc[:], srt[:])
            h2 = []
            for w in range(NCH):
                tp = _t(pool_sq, [128, SHARD], F32, "tmp2")
                nc.vector.tensor_tensor(tp[:], aot[w][:], mur[:], OP.subtract)
                ht = _t(pool_h2, [128, SHARD], BF16, "h2")
                nc.vector.tensor_tensor(ht[:], tp[:], rbc[:], OP.mult)
                h2.append(ht)
            # FC + GELU (stream weights, 8KB/partition descriptors)
            mt = []
            for fg in range(NFB // 4):
                wt = _t(pool_wf, [128, 4, NCH, 128], BF16, "wf")
                nc.sync.dma_start(wt[:], wfc_d.ap()[:, 4 * fg:4 * fg + 4])
                for fs in range(4):
                    f = 4 * fg + fs
                    ps = _t(psd, [128, SHARD], F32, "psd")
                    for cb in range(NCH):
                        nc.tensor.matmul(ps[:], wt[:, fs, cb, :], h2[cb][:],
                                         start=(cb == 0),
                                         stop=(cb == NCH - 1))
                    m = _t(pool_mt, [128, SHARD], BF16, "mt")
                    nc.scalar.activation(m[:], ps[:], AF.Gelu,
                                         bias=bfc_t[:, f:f + 1], scale=1.0)
                    mt.append(m)
            # MP + bias + residual -> out
            for w in range(NCH):
                wt = _t(pool_wm, [128, NFB, 128], BF16, "wm")
                nc.sync.dma_start(wt[:], wmp_d.ap()[:, w])
                ps = _t(psd, [128, SHARD], F32, "psd")
                for f in range(NFB):
                    nc.tensor.matmul(ps[:], wt[:, f, :], mt[f][:],
                                     start=(f == 0), stop=(f == NFB - 1))
                ot = _t(pool_ot, [128, SHARD], F32, "ot")
                nc.vector.scalar_tensor_tensor(
                    out=ot[:], in0=ps[:], scalar=bmp_t[:, w:w + 1],
                    in1=aot[w][:], op0=OP.add, op1=OP.add)
                nc.sync.dma_start(out_d[128 * w:128 * (w + 1), :], ot[:])

    nc.compile()
    return nc


def _prep(inputs):
    """Host-side preprocessing: fold LN affines into weights, relayout."""
    f32 = np.float32
    bf16 = ml_dtypes.bfloat16
    x = np.asarray(inputs["x"], f32).reshape(NTOK, C)
    W_qkv = np.asarray(inputs["W_qkv"], f32)
    b_qkv = np.asarray(inputs["b_qkv"], f32)
    W_ao = np.asarray(inputs["W_ao"], f32)
    b_ao = np.asarray(inputs["b_ao"], f32)
    W_fc = np.asarray(inputs["W_fc"], f32)
    b_fc = np.asarray(inputs["b_fc"], f32)
    W_mp = np.asarray(inputs["W_mp"], f32)
    b_mp = np.asarray(inputs["b_mp"], f32)
    g1 = np.asarray(inputs["g1"], f32)
    be1 = np.asarray(inputs["be1"], f32)
    g2 = np.asarray(inputs["g2"], f32)
    be2 = np.asarray(inputs["be2"], f32)

    Wq_eff = W_qkv * g1[:, None]
    bq_eff = b_qkv + be1 @ W_qkv
    # fold 1/sqrt(D) into the Q columns
    Wq_eff[:, :C] *= 1.0 / np.sqrt(D)
    bq_eff[:C] *= 1.0 / np.sqrt(D)
    Wfc_eff = W_fc * g2[:, None]
    bfc_eff = b_fc + be2 @ W_fc

    xT = np.ascontiguousarray(x.T)  # [C, NTOK]
    # [128, chunk, kb, 512]: partition-major, 8KB contiguous per (p, chunk)
    xTr = np.ascontiguousarray(
        xT.astype(bf16).reshape(NCH, 128, NCHUNK, 512).transpose(1, 2, 0, 3))
    mask = (np.arange(128)[:, None] <= np.arange(128)[None, :]).astype(bf16)
    ident = np.eye(128, dtype=bf16)

    # wao: [128, w, kb, 128]
    waor = np.ascontiguousarray(
        W_ao.astype(bf16).reshape(NCH, 128, NCH, 128).transpose(1, 2, 0, 3))
    # wfc: [128, f, kb, 128]
    wfcr = np.ascontiguousarray(
        Wfc_eff.astype(bf16).reshape(NCH, 128, NFB, 128).transpose(1, 2, 0, 3))
    # wmp: [128, w, fb, 128]
    wmpr = np.ascontiguousarray(
        W_mp.astype(bf16).reshape(NFB, 128, NCH, 128).transpose(1, 2, 0, 3))
    bao_c = np.ascontiguousarray(b_ao.reshape(NCH, 128).T)
    bfc_c = np.ascontiguousarray(bfc_eff.reshape(NFB, 128).T)
    bmp_c = np.ascontiguousarray(b_mp.reshape(NCH, 128).T)

    with_bias_qkv = bool(np.any(bq_eff != 0.0))

    in_maps = []
    for r in range(N_CORES):
        cs = 128 * r
        # per-core qkv column block: [C, 3, 128] -> [128, 3, kb, 128]
        wq_core = np.stack(
            [Wq_eff[:, o * C + cs:o * C + cs + 128] for o in range(3)],
            axis=1)  # [C, 3, 128]
        wq_bf = wq_core.astype(bf16)
        wqkvr = np.ascontiguousarray(
            wq_bf.reshape(NCH, 128, 3, 128).transpose(1, 2, 0, 3))
        # exact colsums of the bf16 weights actually used on device
        wsum_core = wq_bf.astype(f32).sum(axis=0)  # [3, 128]
        bq_core = np.stack(
            [bq_eff[o * C + cs:o * C + cs + 128] for o in range(3)], axis=0)
        # interleaved shard: 256 tokens of batch 0 + 256 of batch 1
        cols = np.concatenate([
            np.arange(HALF * r, HALF * r + HALF),
            np.arange(T + HALF * r, T + HALF * r + HALF)])
        xts_core = np.ascontiguousarray(
            xT[:, cols].reshape(NCH, 128, SHARD).transpose(1, 0, 2))
        in_maps.append({
            "xTr": xTr,
            "xts": xts_core,
            "wqkv": wqkvr,
            "wsum": np.ascontiguousarray(wsum_core.T),
            "bqkv": np.ascontiguousarray(bq_core.T),
            "wao": waor,
            "bao": bao_c,
            "wfc": wfcr,
            "bfc": bfc_c,
            "wmp": wmpr,
            "bmp": bmp_c,
            "mask": mask,
            "ident": ident,
        })
    return in_maps, with_bias_qkv


def kernel(_trace=False, _trace_kwargs=None, **inputs):
    in_maps, with_bias_qkv = _prep(inputs)
    key = ("nc", with_bias_qkv, _DEBUG)
    if key not in _CACHE:
        _CACHE[key] = _build(with_bias_qkv)
    nc = _CACHE[key]
    res = run_bass_kernel_spmd(
        nc, in_maps, core_ids=list(range(N_CORES)),
        trace=_trace, **(_trace_kwargs or {}))
    _CACHE["last_results"] = res
    # un-shard: core r holds [C, 512] = [b0 tokens 256r:256r+256 | b1 same]
    out_full = np.empty((C, NTOK), dtype=np.float32)
    for r in range(N_CORES):
        o = np.asarray(res.results[r]["out"])
        out_full[:, HALF * r:HALF * r + HALF] = o[:, :HALF]
        out_full[:, T + HALF * r:T + HALF * r + HALF] = o[:, HALF:]
    return np.ascontiguousarray(out_full.T.reshape(B, T, C)).astype(np.float32)


# revision 32
# speedup vs baseline: 1.3377x; 1.0272x over previous
"""Trainium2 Bass kernel for a GPT-style transformer block (B=2, T=2048, C=1024, H=16).

Sharding: Megatron-style tensor parallelism over 8 NeuronCores.
  - Attention is head-parallel: each core computes QKV / attention for its 2 heads
    over all 4096 tokens.
  - Two half-size AllToAlls (one per batch) redistribute attention outputs from
    head-sharded to token-sharded. Each core's token shard interleaves both
    batches (256 tokens of batch 0 + 256 of batch 1) so neither collective
    ships zero padding, and the batch-0 half is assembled under batch-1 compute.
  - The output projection, LayerNorm2 and the MLP are token-parallel: each core
    handles its 512-token shard with the full weight matrices.

Everything on-device runs in "transposed" layouts [feature, token]. LayerNorm
affine params are folded into the weights on the host; the LN normalizations
are folded around the matmuls as per-token affines computed from broadcasted
statistics (ones[128,128] stationary matmuls give partition-broadcast sums
directly). LN stats for all 4 chunks of a batch are computed up-front so the
scalar engine's activation table switches (Sqrt vs Exp) happen once per batch
instead of per chunk. All weights are laid out host-side so every DMA
descriptor is >=8KB contiguous per partition.
"""

from contextlib import ExitStack

import numpy as np
import ml_dtypes

import concourse.bass as bass
import concourse.bacc as bacc
import concourse.mybir as mybir
import concourse.tile as tile
from concourse.bass_utils import run_bass_kernel_spmd

BF16 = mybir.dt.bfloat16
F32 = mybir.dt.float32
AF = mybir.ActivationFunctionType
OP = mybir.AluOpType

N_CORES = 8
B, T, C, H, D = 2, 2048, 1024, 16, 64
NTOK = B * T  # 4096
F = 4 * C  # 4096
LN_EPS = 1e-5
HPC = H // N_CORES  # heads per core = 2
SHARD = NTOK // N_CORES  # 512 tokens per core (256 from each batch)
HALF = SHARD // 2  # 256
NCH = C // 128  # 8 channel blocks
NFB = F // 128  # 32 ffn blocks
NCHUNK = NTOK // 512  # 8 token chunks of 512
SB = 128  # s-block size
VW = D + 1  # V block width incl. ones column = 65

_CACHE = {}
_DEBUG = False


def _pbc(ap, n):
    """Partition-broadcast AP: read `ap` (1-D) n times across partitions."""
    return bass.AP(tensor=ap.tensor, offset=ap.offset,
                   ap=[[0, n]] + [list(x) for x in ap.ap])


_UID = [0]


def _t(pool, shape, dtype, tag):
    _UID[0] += 1
    return pool.tile(shape, dtype, tag=tag, name=f"{tag}_{_UID[0]}")


def _build(with_bias_qkv: bool):
    nc = bacc.Bacc("TRN2", target_bir_lowering=False, debug=False,
                   num_devices=N_CORES)

    # ---- I/O ----  (all weight tensors pre-laid-out host side, partition-major)
    xTr_d = nc.dram_tensor("xTr", [128, NCHUNK, NCH, 512], BF16,
                           kind="ExternalInput")
    xts_d = nc.dram_tensor("xts", [128, NCH, SHARD], F32, kind="ExternalInput")
    wqkv_d = nc.dram_tensor("wqkv", [128, 3, NCH, 128], BF16,
                            kind="ExternalInput")
    wsum_d = nc.dram_tensor("wsum", [128, 3], F32, kind="ExternalInput")
    bqkv_d = nc.dram_tensor("bqkv", [128, 3], F32, kind="ExternalInput")
    wao_d = nc.dram_tensor("wao", [128, NCH, NCH, 128], BF16,
                           kind="ExternalInput")
    bao_d = nc.dram_tensor("bao", [128, NCH], F32, kind="ExternalInput")
    wfc_d = nc.dram_tensor("wfc", [128, NFB, NCH, 128], BF16,
                           kind="ExternalInput")
    bfc_d = nc.dram_tensor("bfc", [128, NFB], F32, kind="ExternalInput")
    wmp_d = nc.dram_tensor("wmp", [128, NCH, NFB, 128], BF16,
                           kind="ExternalInput")
    bmp_d = nc.dram_tensor("bmp", [128, NCH], F32, kind="ExternalInput")
    mask_d = nc.dram_tensor("mask", [128, 128], BF16, kind="ExternalInput")
    ident_d = nc.dram_tensor("ident", [128, 128], BF16, kind="ExternalInput")
    out_d = nc.dram_tensor("out", [C, SHARD], F32, kind="ExternalOutput")
    if _DEBUG:
        dbg_q_d = nc.dram_tensor("dbg_q", [128, 512], BF16,
                                 kind="ExternalOutput")
        dbg_yt_d = nc.dram_tensor("dbg_yt", [VW, 512], BF16,
                                  kind="ExternalOutput")
        dbg_ya_d = nc.dram_tensor("dbg_ya", [128, SHARD], BF16,
                                  kind="ExternalOutput")
        dbg_ao_d = nc.dram_tensor("dbg_ao", [128, SHARD], F32,
                                  kind="ExternalOutput")

    with tile.TileContext(nc) as tc, ExitStack() as _es:
        singles = _es.enter_context(tc.tile_pool(name="singles", bufs=1))
        dram = _es.enter_context(tc.tile_pool(name="dram", bufs=1, space="DRAM"))
        # y-assembly pools live across both phases: the batch-0 half is
        # assembled under batch-1 attention compute.
        pool_ya = _es.enter_context(tc.tile_pool(name="ya", bufs=8))
        pool_dn = _es.enter_context(tc.tile_pool(name="dn", bufs=10))
        pool_rdn = _es.enter_context(tc.tile_pool(name="rdn", bufs=2))
        # ---------- constants (tiny DMAs first; big weights deferred) ------
        eps_t = _t(singles, [128, 1], F32, "eps")
        nc.vector.memset(eps_t[:], LN_EPS)
        ones128_t = _t(singles, [128, 128], BF16, "ones128")
        nc.vector.memset(ones128_t[:], 1.0)
        mask_t = _t(singles, [128, 128], BF16, "mask")
        nc.sync.dma_start(mask_t[:], mask_d[:])
        ident_t = _t(singles, [128, 128], BF16, "ident")
        nc.sync.dma_start(ident_t[:], ident_d[:])
        wsum_t = _t(singles, [128, 3], F32, "wsum")
        nc.sync.dma_start(wsum_t[:], wsum_d[:])
        bqkv_t = _t(singles, [128, 3], F32, "bqkv")
        nc.sync.dma_start(bqkv_t[:], bqkv_d[:])
        bao_t = _t(singles, [128, NCH], F32, "bao")
        nc.sync.dma_start(bao_t[:], bao_d[:])
        bfc_t = _t(singles, [128, NFB], F32, "bfc")
        nc.sync.dma_start(bfc_t[:], bfc_d[:])
        bmp_t = _t(singles, [128, NCH], F32, "bmp")
        nc.sync.dma_start(bmp_t[:], bmp_d[:])
        # big resident weights; DMAs issued later to keep startup lean
        wqkv_t = _t(singles, [128, 3, NCH, 128], BF16, "wqkv")
        wao_t = _t(singles, [128, NCH, NCH, 128], BF16, "wao")

        # A2A dram buffers: per batch, slot i carries the 256-token half-shard
        # i (tokens [256*i, 256*i+256) of that batch) for this core's 2 heads:
        # 64 raw y rows + the softmax denominator row. No zero slots.
        a2a_in1 = _t(dram, [N_CORES, HPC, VW, HALF], BF16, "a2a_in1")
        a2a_out1 = _t(dram, [N_CORES, HPC, VW, HALF], BF16, "a2a_out1")
        a2a_in2 = _t(dram, [N_CORES, HPC, VW, HALF], BF16, "a2a_in2")
        a2a_out2 = _t(dram, [N_CORES, HPC, VW, HALF], BF16, "a2a_out2")

        yall = [None] * N_CORES
        yhalves = {}

        def fetch_half(i, half, a2a_out):
            """DMA source core i's raw y + broadcast denominators (one half).

            Pure DMA: safe to issue right after the collective without
            blocking any compute engine's in-order queue. Raw y lands
            directly in yall[i]; the normalize multiplies it in place.
            """
            if yall[i] is None:
                yall[i] = _t(pool_ya, [128, SHARD], BF16, "ya")
            c0 = 0 if half == 0 else HALF
            dn = _t(pool_dn, [128, HALF], F32, "dn")
            for h in range(HPC):
                nc.sync.dma_start(yall[i][64 * h:64 * (h + 1), c0:c0 + HALF],
                                  a2a_out[i, h, 0:D, :])
                nc.gpsimd.dma_start(dn[64 * h:64 * (h + 1), :],
                                    _pbc(a2a_out[i, h, D, :], D))
            yhalves[i, half] = dn

        def norm_half(i, half):
            """Normalize one fetched half of yall[i] in place (vector)."""
            c0 = 0 if half == 0 else HALF
            dn = yhalves.pop((i, half))
            rdn = _t(pool_rdn, [128, HALF], F32, "rdn")
            nc.vector.reciprocal_approx_fast(rdn[:], dn[:])
            ysl = yall[i][:, c0:c0 + HALF]
            nc.vector.tensor_tensor(ysl, ysl, rdn[:], OP.mult)

        # ---- Phase ABC: LN1 stats + QKV + attention, batch-pipelined ----
        with ExitStack() as es1:
            ps_sh = es1.enter_context(
                tc.tile_pool(name="ps_sh", bufs=2, space="PSUM"))
            pscp = es1.enter_context(
                tc.tile_pool(name="ps_c", bufs=2, space="PSUM"))
            psyp = es1.enter_context(
                tc.tile_pool(name="ps_y", bufs=2, space="PSUM"))
            pool_xt = es1.enter_context(tc.tile_pool(name="xt", bufs=5))
            pool_st = es1.enter_context(tc.tile_pool(name="st", bufs=4))
            pool_ab = es1.enter_context(tc.tile_pool(name="ab", bufs=5))
            pool_sqx = es1.enter_context(tc.tile_pool(name="sqx", bufs=4))
            pool_tmp = es1.enter_context(tc.tile_pool(name="tmpq", bufs=2))
            pool_vt = es1.enter_context(tc.tile_pool(name="vt", bufs=2))
            pool_att = es1.enter_context(tc.tile_pool(name="att", bufs=4))
            pool_yt = es1.enter_context(tc.tile_pool(name="yt", bufs=4))
            pool_qkvT = es1.enter_context(tc.tile_pool(name="qkvT", bufs=1))

            # attention result tiles (live through phase ABC only)
            qT_t = _t(pool_qkvT, [128, NTOK], BF16, "qT")
            kT_t = _t(pool_qkvT, [128, NTOK], BF16, "kT")
            vT_t = _t(pool_qkvT, [128, NTOK], BF16, "vT")
            qkv_dest = [qT_t, kT_t, vT_t]

            vt = {}
            xt_tiles = {}
            ab_tiles = {}

            def stats_chunk(b, j):
                g = 4 * b + j
                # ---- load x chunk (fat descriptors: 8KB/partition) ----
                xt = _t(pool_xt, [128, NCH, 512], BF16, "xt")
                nc.sync.dma_start(xt[:], xTr_d.ap()[:, g])
                xt_tiles[g] = xt
                if b == 0 and j == 0:
                    # big weight prefetches ride behind the first chunk
                    nc.sync.dma_start(wqkv_t[:], wqkv_d[:])
                if b == 0 and j == 3:
                    nc.sync.dma_start(wao_t[:], wao_d[:])
                # ---- LN1 stats via broadcast matmuls (M=128) ----
                s1p = _t(ps_sh, [128, 512], F32, "ps")
                for kb in range(NCH):
                    nc.tensor.matmul(s1p[:], ones128_t[:], xt[:, kb, :],
                                     start=(kb == 0), stop=(kb == NCH - 1))
                s2p = _t(ps_sh, [128, 512], F32, "ps")
                for kb in range(NCH):
                    sq = _t(pool_sqx, [128, 512], BF16, "sqx")
                    nc.vector.tensor_tensor(sq[:], xt[:, kb, :],
                                            xt[:, kb, :], OP.mult)
                    nc.tensor.matmul(s2p[:], ones128_t[:], sq[:],
                                     start=(kb == 0), stop=(kb == NCH - 1))
                # broadcast rows: nmu = -mean, abc = 1/sqrt(var+eps)
                nmu = _t(pool_ab, [128, 512], BF16, "nmu")
                nc.vector.tensor_scalar(out=nmu[:], in0=s1p[:],
                                        scalar1=-1.0 / C, scalar2=0.0,
                                        op0=OP.mult, op1=OP.add)
                mur = _t(pool_st, [128, 512], F32, "mur")
                nc.vector.tensor_scalar(out=mur[:], in0=s1p[:],
                                        scalar1=1.0 / C, scalar2=0.0,
                                        op0=OP.mult, op1=OP.add)
                msq = _t(pool_st, [128, 512], F32, "msq")
                nc.vector.tensor_tensor(msq[:], mur[:], mur[:], OP.mult)
                var = _t(pool_st, [128, 512], F32, "var")
                nc.vector.scalar_tensor_tensor(
                    out=var[:], in0=s2p[:], scalar=1.0 / C, in1=msq[:],
                    op0=OP.mult, op1=OP.subtract)
                srt = _t(pool_st, [128, 512], F32, "srt")
                nc.scalar.activation(srt[:], var[:], AF.Sqrt,
                                     bias=eps_t[:], scale=1.0)
                abc = _t(pool_ab, [128, 512], F32, "abc")
                nc.vector.reciprocal_approx_fast(abc[:], srt[:])
                ab_tiles[g] = (nmu, abc)

            def att_chunk(b, j):
                g = 4 * b + j
                t0 = 512 * g
                # ---- QKV on raw x + rank-1 LN correction on vector ----
                xt = xt_tiles.pop(g)
                nmu, abc = ab_tiles.pop(g)
                for o in range(3):
                    ps = _t(ps_sh, [128, 512], F32, "ps")
                    for kb in range(NCH):
                        nc.tensor.matmul(ps[:], wqkv_t[:, o, kb, :],
                                         xt[:, kb, :],
                                         start=(kb == 0), stop=(kb == NCH - 1))
                    tmp = _t(pool_tmp, [128, 512], F32, "tmpq")
                    nc.vector.scalar_tensor_tensor(
                        out=tmp[:], in0=nmu[:], scalar=wsum_t[:, o:o + 1],
                        in1=ps[:], op0=OP.mult, op1=OP.add)
                    dst = qkv_dest[o][:, t0:t0 + 512]
                    nc.vector.tensor_tensor(dst, tmp[:], abc[:], OP.mult)
                    if _DEBUG and g == 0 and o == 0:
                        nc.sync.dma_start(dbg_q_d[:], dst)
                    if with_bias_qkv:
                        nc.vector.tensor_scalar(
                            out=dst, in0=dst,
                            scalar1=bqkv_t[:, o:o + 1], scalar2=0.0,
                            op0=OP.add, op1=OP.add)
                # ---- attention for this chunk ----
                nblk = 4 * j + 4
                if j == 0:
                    for h in range(HPC):
                        v = _t(pool_vt, [128, T // SB, VW], BF16, f"vt{h}")
                        nc.vector.memset(v[:, :, D:VW], 1.0)
                        vt[b, h] = v
                # transpose V for the newly available s-blocks
                for i in range(4 * j, 4 * j + 4):
                    s0 = 2048 * b + SB * i
                    for h in range(HPC):
                        pst = _t(ps_sh, [128, D], BF16, "ps")
                        nc.tensor.transpose(
                            pst[:],
                            vT_t[64 * h:64 * (h + 1), s0:s0 + SB],
                            ident_t[64 * h:64 * (h + 1),
                                    64 * h:64 * (h + 1)],
                        )
                        nc.vector.tensor_copy(vt[b, h][:, i, 0:D], pst[:])
                psy = [_t(psyp, [VW, 512], F32, "psy") for h in range(HPC)]
                for i in range(nblk):
                    s0 = 2048 * b + SB * i
                    m = i - 4 * j  # >= 0 on diagonal blocks
                    f0 = 128 * m if m >= 0 else 0  # causal: t-f0 cols only
                    psc = _t(pscp, [128, HPC, 512], F32, "psc")
                    for h in range(HPC):
                        nc.tensor.matmul(
                            psc[:, h, 0:512 - f0],
                            kT_t[64 * h:64 * (h + 1), s0:s0 + SB],
                            qT_t[64 * h:64 * (h + 1), t0 + f0:t0 + 512],
                            tile_position=(64 * h, 0),
                        )
                    at = _t(pool_att, [128, HPC, 512], BF16, "att")
                    nc.scalar.activation(at[:, :, 0:512 - f0],
                                         psc[:, :, 0:512 - f0], AF.Exp)
                    if m >= 0:  # diagonal: mask boundary block only
                        for h in range(HPC):
                            nc.vector.tensor_tensor(
                                at[:, h, 0:128], at[:, h, 0:128],
                                mask_t[:], OP.mult)
                    for h in range(HPC):
                        nc.tensor.matmul(
                            psy[h][:, f0:512], vt[b, h][:, i, :],
                            at[:, h, 0:512 - f0],
                            start=(i == 0), stop=(i == nblk - 1))
                # ship raw y + denominator row; normalize post-A2A
                a2a_in = a2a_in1 if b == 0 else a2a_in2
                for h in range(HPC):
                    yt = _t(pool_yt, [VW, 512], BF16, "yt")
                    nc.vector.tensor_copy(yt[:], psy[h][:])
                    if _DEBUG and g == 0 and h == 0:
                        nc.sync.dma_start(dbg_yt_d[:], yt[:])
                    nc.sync.dma_start(a2a_in[2 * j, h], yt[:, 0:HALF])
                    nc.sync.dma_start(a2a_in[2 * j + 1, h], yt[:, HALF:512])

            for b in range(B):
                for j in range(4):
                    stats_chunk(b, j)
                for j in range(4):
                    att_chunk(b, j)
                nc.gpsimd.collective_compute(
                    "AllToAll", OP.bypass,
                    replica_groups=[list(range(N_CORES))],
                    ins=[(a2a_in1 if b == 0 else a2a_in2).opt()],
                    outs=[(a2a_out1 if b == 0 else a2a_out2).opt()],
                )
                if b == 0:
                    # fetch the batch-0 half under batch-1 compute (DMA only)
                    for i in range(N_CORES):
                        fetch_half(i, 0, a2a_out1)

        # ---------- Phase D: AO proj + LN2 + MLP on the token shard ----
        with ExitStack() as es3:
            psd = es3.enter_context(
                tc.tile_pool(name="psd", bufs=4, space="PSUM"))
            psr = es3.enter_context(
                tc.tile_pool(name="psr", bufs=2, space="PSUM"))
            pool_x3 = es3.enter_context(tc.tile_pool(name="x3", bufs=1))
            pool_ao = es3.enter_context(tc.tile_pool(name="ao", bufs=8))
            pool_aob = es3.enter_context(tc.tile_pool(name="aob", bufs=8))
            pool_sq = es3.enter_context(tc.tile_pool(name="sq", bufs=2))
            pool_st2 = es3.enter_context(tc.tile_pool(name="st2", bufs=4))
            pool_h2 = es3.enter_context(tc.tile_pool(name="h2", bufs=8))
            pool_mt = es3.enter_context(tc.tile_pool(name="mt", bufs=NFB))
            pool_wf = es3.enter_context(tc.tile_pool(name="wf", bufs=2))
            pool_wm = es3.enter_context(tc.tile_pool(name="wm", bufs=2))
            pool_ot = es3.enter_context(tc.tile_pool(name="ot", bufs=2))

            xts = _t(pool_x3, [128, NCH, SHARD], F32, "xts")
            nc.sync.dma_start(xts[:], xts_d[:])
            # batch-1 half DMAs, then normalize: batch-0 halves have their
            # data already, so those vector ops run during the collective
            for i in range(N_CORES):
                fetch_half(i, 1, a2a_out2)
            for i in range(N_CORES):
                norm_half(i, 0)
            for i in range(N_CORES):
                norm_half(i, 1)
            if _DEBUG:
                nc.sync.dma_start(dbg_ya_d[:], yall[0][:])
            aot, aob = [], []
            for w in range(NCH):
                ps = _t(psd, [128, SHARD], F32, "psd")
                for i in range(NCH):
                    nc.tensor.matmul(ps[:], wao_t[:, w, i, :], yall[i][:],
                                     start=(i == 0), stop=(i == NCH - 1))
                ao = _t(pool_ao, [128, SHARD], F32, "ao")
                nc.vector.scalar_tensor_tensor(
                    out=ao[:], in0=ps[:], scalar=bao_t[:, w:w + 1],
                    in1=xts[:, w, :], op0=OP.add, op1=OP.add)
                ab = _t(pool_aob, [128, SHARD], BF16, "aob")
                nc.vector.tensor_copy(ab[:], ao[:])
                if _DEBUG and w == 0:
                    nc.sync.dma_start(dbg_ao_d[:], ao[:])
                aot.append(ao)
                aob.append(ab)
            # LN2 stats via broadcast matmuls
            ps1 = _t(psr, [128, SHARD], F32, "psr")
            for w in range(NCH):
                nc.tensor.matmul(ps1[:], ones128_t[:], aob[w][:],
                                 start=(w == 0), stop=(w == NCH - 1))
            ps2 = _t(psr, [128, SHARD], F32, "psr")
            for w in range(NCH):
                sq = _t(pool_sq, [128, SHARD], BF16, "sq")
                nc.vector.tensor_tensor(sq[:], aob[w][:], aob[w][:],
                                        OP.mult)
                nc.tensor.matmul(ps2[:], ones128_t[:], sq[:],
                                 start=(w == 0), stop=(w == NCH - 1))
            mur = _t(pool_st2, [128, SHARD], F32, "mur2")
            nc.vector.tensor_scalar(out=mur[:], in0=ps1[:], scalar1=1.0 / C,
                                    scalar2=0.0, op0=OP.mult, op1=OP.add)
            msq = _t(pool_st2, [128, SHARD], F32, "msq2")
            nc.vector.tensor_tensor(msq[:], mur[:], mur[:], OP.mult)
            var = _t(pool_st2, [128, SHARD], F32, "var2")
            nc.vector.scalar_tensor_tensor(
                out=var[:], in0=ps2[:], scalar=1.0 / C, in1=msq[:],
                op0=OP.mult, op1=OP.subtract)
            srt = _t(pool_st2, [128, SHARD], F32, "srt2")
            nc.scalar.activation(srt[:], var[:], AF.Sqrt,
                                 bias=eps_t[:], scale=1.0)
            rbc = _t(pool_st2, [128, SHARD], F32, "rbc2")
            nc.vector.reciprocal_approx_fast(rbc[:], srt[:])
            h2 = []
            for w in range(NCH):
                tp = _t(pool_sq, [128, SHARD], F32, "tmp2")
                nc.vector.tensor_tensor(tp[:], aot[w][:], mur[:], OP.subtract)
                ht = _t(pool_h2, [128, SHARD], BF16, "h2")
                nc.vector.tensor_tensor(ht[:], tp[:], rbc[:], OP.mult)
                h2.append(ht)
            # FC + GELU (stream weights, 8KB/partition descriptors)
            mt = []
            for fg in range(NFB // 4):
                wt = _t(pool_wf, [128, 4, NCH, 128], BF16, "wf")
                nc.sync.dma_start(wt[:], wfc_d.ap()[:, 4 * fg:4 * fg + 4])
                for fs in range(4):
                    f = 4 * fg + fs
                    ps = _t(psd, [128, SHARD], F32, "psd")
                    for cb in range(NCH):
                        nc.tensor.matmul(ps[:], wt[:, fs, cb, :], h2[cb][:],
                                         start=(cb == 0),
                                         stop=(cb == NCH - 1))
                    m = _t(pool_mt, [128, SHARD], BF16, "mt")
                    nc.scalar.activation(m[:], ps[:], AF.Gelu,
                                         bias=bfc_t[:, f:f + 1], scale=1.0)
                    mt.append(m)
            # MP + bias + residual -> out
            for w in range(NCH):
                wt = _t(pool_wm, [128, NFB, 128], BF16, "wm")
                nc.sync.dma_start(wt[:], wmp_d.ap()[:, w])
                ps = _t(psd, [128, SHARD], F32, "psd")
                for f in range(NFB):
                    nc.tensor.matmul(ps[:], wt[:, f, :], mt[f][:],
                                     start=(f == 0), stop=(f == NFB - 1))
                ot = _t(pool_ot, [128, SHARD], F32, "ot")
                nc.vector.scalar_tensor_tensor(
                    out=ot[:], in0=ps[:], scalar=bmp_t[:, w:w + 1],
                    in1=aot[w][:], op0=OP.add, op1=OP.add)
                nc.sync.dma_start(out_d[128 * w:128 * (w + 1), :], ot[:])

    nc.compile()
    return nc


def _prep(inputs):
    """Host-side preprocessing: fold LN affines into weights, relayout."""
    f32 = np.float32
    bf16 = ml_dtypes.bfloat16
    x = np.asarray(inputs["x"], f32).reshape(NTOK, C)
    W_qkv = np.asarray(inputs["W_qkv"], f32)
    b_qkv = np.asarray(inputs["b_qkv"], f32)
    W_ao = np.asarray(inputs["W_ao"], f32)
    b_ao = np.asarray(inputs["b_ao"], f32)
    W_fc = np.asarray(inputs["W_fc"], f32)
    b_fc = np.asarray(inputs["b_fc"], f32)
    W_mp = np.asarray(inputs["W_mp"], f32)
    b_mp = np.asarray(inputs["b_mp"], f32)
    g1 = np.asarray(inputs["g1"], f32)
    be1 = np.asarray(inputs["be1"], f32)
    g2 = np.asarray(inputs["g2"], f32)
    be2 = np.asarray(inputs["be2"], f32)

    Wq_eff = W_qkv * g1[:, None]
    bq_eff = b_qkv + be1 @ W_qkv
    # fold 1/sqrt(D) into the Q columns
    Wq_eff[:, :C] *= 1.0 / np.sqrt(D)
    bq_eff[:C] *= 1.0 / np.sqrt(D)
    Wfc_eff = W_fc * g2[:, None]
    bfc_eff = b_fc + be2 @ W_fc

    xT = np.ascontiguousarray(x.T)  # [C, NTOK]
    # [128, chunk, kb, 512]: partition-major, 8KB contiguous per (p, chunk)
    xTr = np.ascontiguousarray(
        xT.astype(bf16).reshape(NCH, 128, NCHUNK, 512).transpose(1, 2, 0, 3))
    mask = (np.arange(128)[:, None] <= np.arange(128)[None, :]).astype(bf16)
    ident = np.eye(128, dtype=bf16)

    # wao: [128, w, kb, 128]
    waor = np.ascontiguousarray(
        W_ao.astype(bf16).reshape(NCH, 128, NCH, 128).transpose(1, 2, 0, 3))
    # wfc: [128, f, kb, 128]
    wfcr = np.ascontiguousarray(
        Wfc_eff.astype(bf16).reshape(NCH, 128, NFB, 128).transpose(1, 2, 0, 3))
    # wmp: [128, w, fb, 128]
    wmpr = np.ascontiguousarray(
        W_mp.astype(bf16).reshape(NFB, 128, NCH, 128).transpose(1, 2, 0, 3))
    bao_c = np.ascontiguousarray(b_ao.reshape(NCH, 128).T)
    bfc_c = np.ascontiguousarray(bfc_eff.reshape(NFB, 128).T)
    bmp_c = np.ascontiguousarray(b_mp.reshape(NCH, 128).T)

    with_bias_qkv = bool(np.any(bq_eff != 0.0))

    in_maps = []
    for r in range(N_CORES):
        cs = 128 * r
        # per-core qkv column block: [C, 3, 128] -> [128, 3, kb, 128]
        wq_core = np.stack(
            [Wq_eff[:, o * C + cs:o * C + cs + 128] for o in range(3)],
            axis=1)  # [C, 3, 128]
        wq_bf = wq_core.astype(bf16)
        wqkvr = np.ascontiguousarray(
            wq_bf.reshape(NCH, 128, 3, 128).transpose(1, 2, 0, 3))
        # exact colsums of the bf16 weights actually used on device
        wsum_core = wq_bf.astype(f32).sum(axis=0)  # [3, 128]
        bq_core = np.stack(
            [bq_eff[o * C + cs:o * C + cs + 128] for o in range(3)], axis=0)
        # interleaved shard: 256 tokens of batch 0 + 256 of batch 1
        cols = np.concatenate([
            np.arange(HALF * r, HALF * r + HALF),
            np.arange(T + HALF * r, T + HALF * r + HALF)])
        xts_core = np.ascontiguousarray(
            xT[:, cols].reshape(NCH, 128, SHARD).transpose(1, 0, 2))
        in_maps.append({
            "xTr": xTr,
            "xts": xts_core,
            "wqkv": wqkvr,
            "wsum": np.ascontiguousarray(wsum_core.T),
            "bqkv": np.ascontiguousarray(bq_core.T),
            "wao": waor,
            "bao": bao_c,
            "wfc": wfcr,
            "bfc": bfc_c,
            "wmp": wmpr,
            "bmp": bmp_c,
            "mask": mask,
            "ident": ident,
        })
    return in_maps, with_bias_qkv


def kernel(_trace=False, _trace_kwargs=None, **inputs):
    in_maps, with_bias_qkv = _prep(inputs)
    key = ("nc", with_bias_qkv, _DEBUG)
    if key not in _CACHE:
        _CACHE[key] = _build(with_bias_qkv)
    nc = _CACHE[key]
    res = run_bass_kernel_spmd(
        nc, in_maps, core_ids=list(range(N_CORES)),
        trace=_trace, **(_trace_kwargs or {}))
    _CACHE["last_results"] = res
    # un-shard: core r holds [C, 512] = [b0 tokens 256r:256r+256 | b1 same]
    out_full = np.empty((C, NTOK), dtype=np.float32)
    for r in range(N_CORES):
        o = np.asarray(res.results[r]["out"])
        out_full[:, HALF * r:HALF * r + HALF] = o[:, :HALF]
        out_full[:, T + HALF * r:T + HALF * r + HALF] = o[:, HALF:]
    return np.ascontiguousarray(out_full.T.reshape(B, T, C)).astype(np.float32)
